# revision 3
# baseline (speedup 1.0000x reference)
"""CrossAttention2D Trainium2 kernel (v3: scheduler-priority + startup/tail restructure).

Sharding: data-parallel over batch. B=8 -> one batch element per NeuronCore,
no collectives. Weights replicated; host pre-transposes and casts to bf16.

Per-core math (C=512, Ccross=768, N=1024, 8 heads x 64):
  Q = Wq @ x_b          [C, N]   bf16
  K = Wk @ y_b          [C, N]   bf16
  VTa = [(Wv @ y_b).T | 1]       [N, 8*(64+1)] bf16 (ones col per head)
  per head pair ph (heads at PE rows 0/64, row-tiled scores):
    S[k, q] = K_h^T Q_h          psum [128, 1024] per (half, kt), ping-pong
    ET = exp(S/8)                ACT -> SBUF bf16, resident for whole pair
    O_aug[q, 0:65] = ET_tile^T @ VTa_h   (ET stationary: 65-col streams,
                                          accumulated over kt; col 64 = denom)
    O = O_aug[:, :64] / O_aug[:, 64]     (DVE reciprocal + scale) -> bf16
  quirk: out_flat[h*64 + q//16, 64*(q%16) + d] = O_h[q, d]  (DMA shuffle)
  out = Wo @ quirk + bo          [C, N] -> bf16 out, host casts to f32

v3 scheduling (driven by the ntff trace of v2):
  - the exp-critical chain (Q/K proj, score MMs, exp) runs under
    tc.high_priority so score MMs are never buried behind AV bursts in the
    PE queue (v2 lost ~1us of ACT per pair boundary to this)
  - units are ordered half-outer (all half0 exps, then half1), so the AV
    cells of bank0 (which only need half0 ETs) start mid-pair and the
    last pair's tail is just bank1's AV
  - input DMAs are sliced (x/y by column half, wq/wk ct0 first) and spread
    across the two HWDGE queues (sync + scalar) so the first exp fires ~7us
  - quirk/out DMAs of the last pair alternate sync/scalar (ACT idle then)
  - out-proj bias adds alternate DVE/ACT in the tail
"""

import numpy as np
import ml_dtypes

import concourse.bass as bass
import concourse.mybir as mybir
import concourse.tile as tile
from concourse import bacc
from concourse.bass_utils import run_bass_kernel_spmd

P = 128
C = 512          # d_embed
CC = 768         # d_cross
N = 1024         # H*W = 32*32
NH = 8
DH = 64
CT = C // P      # 4
CCT = CC // P    # 6
QT = N // P      # 8
HW = 32
B = 8
F32 = mybir.dt.float32
BF16 = mybir.dt.bfloat16
HIPRI = 1_000_000

_CACHE = {}


def _build_nc():
    nc = bacc.Bacc("TRN2", target_bir_lowering=False, debug=False, num_devices=B)

    x = nc.dram_tensor("x", [C, N], BF16, kind="ExternalInput")
    y = nc.dram_tensor("y", [CC, N], BF16, kind="ExternalInput")
    wqT = nc.dram_tensor("wqT", [C, C], BF16, kind="ExternalInput")
    wkT = nc.dram_tensor("wkT", [CC, C], BF16, kind="ExternalInput")
    wvT = nc.dram_tensor("wvT", [CC, C], BF16, kind="ExternalInput")
    woT = nc.dram_tensor("woT", [C, C], BF16, kind="ExternalInput")
    bq = nc.dram_tensor("bq", [C], F32, kind="ExternalInput")
    bk = nc.dram_tensor("bk", [C], F32, kind="ExternalInput")
    bv = nc.dram_tensor("bv", [C], BF16, kind="ExternalInput")
    bo = nc.dram_tensor("bo", [C], F32, kind="ExternalInput")
    out = nc.dram_tensor("out", [C, N], BF16, kind="ExternalOutput")

    EXP = mybir.ActivationFunctionType.Exp
    COPY = mybir.ActivationFunctionType.Copy

    with tile.TileContext(nc) as tc:
        with (
            tc.tile_pool(name="const", bufs=1) as constp,
            tc.tile_pool(name="big", bufs=1) as bigp,
            tc.tile_pool(name="et", bufs=16) as etp,
            tc.tile_pool(name="oa", bufs=3) as oap,
            tc.tile_pool(name="ev", bufs=3) as evp,
            tc.tile_pool(name="rcp", bufs=4) as rcpp,
            tc.tile_pool(name="psS", bufs=2, space="PSUM") as psS,
            tc.tile_pool(name="psAV", bufs=2, space="PSUM") as psAV,
            tc.tile_pool(name="psP", bufs=2, space="PSUM") as psP,
        ):
            # ---- constants ----
            ones_r = constp.tile([1, P], BF16, name="ones_r", tag="ones_r")
            nc.vector.memset(ones_r[:], 1.0)
            # preload the exp table set early so the ~2.7us ACT_TABLE_LOAD
            # overlaps the input DMA phase instead of the first real exp
            dmy = constp.tile([P, 1], F32, name="dmy", tag="dmy")
            nc.vector.memset(dmy[:], 0.0)
            dmy2 = constp.tile([P, 1], F32, name="dmy2", tag="dmy2")
            nc.scalar.activation(dmy2[:], dmy[:], EXP)

            bq_sb = constp.tile([P, CT], F32, name="bq", tag="bq")
            nc.sync.dma_start(bq_sb[:], bq.rearrange("(o p) -> p o", p=P))
            bk_sb = constp.tile([P, CT], F32, name="bk", tag="bk")
            nc.sync.dma_start(bk_sb[:], bk.rearrange("(o p) -> p o", p=P))
            bo_sb = constp.tile([P, CT], F32, name="bo", tag="bo")
            nc.sync.dma_start(bo_sb[:], bo.rearrange("(o p) -> p o", p=P))
            bv_sb = constp.tile([1, C], BF16, name="bv", tag="bv")
            nc.sync.dma_start(bv_sb[:], bv[None, :])

            # ---- weight / activation loads, ordered for earliest first exp ----
            x3 = x.rearrange("(t p) n -> p t n", p=P)
            y3 = y.rearrange("(t p) n -> p t n", p=P)
            wq3 = wqT.rearrange("(t p) m -> p t m", p=P)
            wk3 = wkT.rearrange("(t p) m -> p t m", p=P)
            wv3 = wvT.rearrange("(t p) m -> p t m", p=P)
            wo3 = woT.rearrange("(t p) m -> p t m", p=P)

            x_sb = [bigp.tile([P, N], BF16, name=f"x{t}", tag=f"x{t}") for t in range(CT)]
            y_sb = [bigp.tile([P, N], BF16, name=f"y{t}", tag=f"y{t}") for t in range(CCT)]
            wq_sb = [bigp.tile([P, C], BF16, name=f"wq{t}", tag=f"wq{t}") for t in range(CT)]
            wk_sb = [bigp.tile([P, C], BF16, name=f"wk{t}", tag=f"wk{t}") for t in range(CCT)]
            wv_sb = [bigp.tile([P, C], BF16, name=f"wv{t}", tag=f"wv{t}") for t in range(CCT)]
            wo_sb = [bigp.tile([P, C], BF16, name=f"wo{t}", tag=f"wo{t}") for t in range(CT)]

            dmae = [nc.sync, nc.scalar]
            dmai = [0]

            def dma(dst, src):
                dmae[dmai[0] % 2].dma_start(dst, src)
                dmai[0] += 1

            # critical path to exp#1: x half0, wq ct0, y half0, wk ct0
            for t in range(CT):
                dma(x_sb[t][:, 0:512], x3[:, t, 0:512])
            for t in range(CT):
                dma(wq_sb[t][:, 0:P], wq3[:, t, 0:P])
            for t in range(CCT):
                dma(y_sb[t][:, 0:512], y3[:, t, 0:512])
            for t in range(CCT):
                dma(wk_sb[t][:, 0:P], wk3[:, t, 0:P])
            # second wave: the other halves (needed from unit kt=4 / half=1)
            for t in range(CT):
                dma(x_sb[t][:, 512:N], x3[:, t, 512:N])
            for t in range(CCT):
                dma(y_sb[t][:, 512:N], y3[:, t, 512:N])
            # remaining weight columns + V/O weights
            for t in range(CT):
                dma(wq_sb[t][:, P:C], wq3[:, t, P:C])
            for t in range(CCT):
                dma(wk_sb[t][:, P:C], wk3[:, t, P:C])
            for t in range(CCT):
                dma(wv_sb[t][:], wv3[:, t])
            for t in range(CT):
                dma(wo_sb[t][:], wo3[:, t])

            q_sb = [bigp.tile([P, N], BF16, name=f"q{t}", tag=f"q{t}") for t in range(CT)]
            k_sb = [bigp.tile([P, N], BF16, name=f"k{t}", tag=f"k{t}") for t in range(CT)]
            # VTa buffer: per n-tile, cols laid out [h][65] with col h*65+64 == 1.0
            vt_sb = [bigp.tile([P, NH * (DH + 1)], BF16, name=f"vt{t}", tag=f"vt{t}")
                     for t in range(QT)]
            for t in range(QT):
                nc.gpsimd.memset(vt_sb[t][:], 1.0)

            # ---- projection helpers ----
            def qk_proj_half(ct, dst, w_tiles, src_tiles, nkt, bias_sb, half):
                ps = psP.tile([P, 512], F32, name="ps", tag="psp")
                for kt in range(nkt):
                    nc.tensor.matmul(
                        ps[:],
                        w_tiles[kt][:, ct * P:(ct + 1) * P],
                        src_tiles[kt][:, half * 512:(half + 1) * 512],
                        start=(kt == 0),
                        stop=(kt == nkt - 1),
                    )
                nc.vector.tensor_scalar_add(
                    dst[:, half * 512:(half + 1) * 512], ps[:], bias_sb[:, ct:ct + 1]
                )

            # ---- VT projection: VT[n, c] = sum_k y[k, n] * wvT[k, c]  (+ bias row)
            def vt_proj(nt):
                ps = psP.tile([P, 512], F32, name="ps", tag="psp")
                for kt in range(CCT):
                    nc.tensor.matmul(
                        ps[:],
                        y_sb[kt][:, nt * P:(nt + 1) * P],
                        wv_sb[kt][:],
                        start=(kt == 0),
                        stop=False,
                    )
                nc.tensor.matmul(ps[:], ones_r[:], bv_sb[:], start=False, stop=True)
                # scatter into [h][0:64] slots (col h*65+64 stays 1.0)
                nc.vector.tensor_copy(
                    out=vt_sb[nt].rearrange("p (h e) -> p h e", e=DH + 1)[:, :, 0:DH],
                    in_=ps.rearrange("p (h d) -> p h d", d=DH),
                )

            with tc.high_priority(offset=HIPRI):
                qk_proj_half(0, q_sb[0], wq_sb, x_sb, CT, bq_sb, 0)
                qk_proj_half(0, k_sb[0], wk_sb, y_sb, CCT, bk_sb, 0)
                qk_proj_half(0, q_sb[0], wq_sb, x_sb, CT, bq_sb, 1)
                qk_proj_half(0, k_sb[0], wk_sb, y_sb, CCT, bk_sb, 1)

            # ---- attention ----
            qk_sb = [bigp.tile([P, N], BF16, name=f"qk{t}", tag=f"qk{t}")
                     for t in range(CT)]

            def ecol(hh, qt):
                return (qt // 4) * 1024 + hh * 512 + (qt % 4) * P

            def emit_av_bank(ph, hh, ets, bank, quirk_spread):
                """AV for one (head, 4-qt bank). Cell accumulation groups within
                one PSUM bank must be sequential (start=True clears has_written
                for the whole bank), so cells run kt-inner back-to-back."""
                h = 2 * ph + hh
                oa = oap.tile([P, 256], BF16, name="oa", tag="oa")
                av = psAV.tile([P, 512], F32, name="av", tag="av")
                for qq in range(4):
                    qt = bank * 4 + qq
                    for kt in range(QT):
                        nc.tensor.matmul(
                            av[:, qq * P:qq * P + DH + 1],
                            ets[kt][:, ecol(hh, qt):ecol(hh, qt) + P],
                            vt_sb[kt][:, h * (DH + 1):(h + 1) * (DH + 1)],
                            start=(kt == 0),
                            stop=(kt == QT - 1),
                        )
                # batched normalize: one reciprocal for the bank's 4
                # denominators, then per-cell scale + shuffle
                rcp = rcpp.tile([P, 4], F32, name="rcp", tag="rcp")
                nc.vector.reciprocal(
                    rcp[:], av.rearrange("p (q c) -> p q c", c=P)[:, :, DH]
                )
                for qq in range(4):
                    qt = bank * 4 + qq
                    nc.vector.tensor_scalar_mul(
                        oa[:, qq * DH:(qq + 1) * DH],
                        av[:, qq * P:qq * P + DH], rcp[:, qq:qq + 1],
                    )
                    # quirk shuffle:
                    # qk[ph][hh*64 + qt*8 + p//16, 64*(p%16)+d] = O_h[qt*128+p, d]
                    eng = dmae[qq % 2] if quirk_spread else nc.sync
                    eng.dma_start(
                        qk_sb[ph][hh * 64 + qt * 8: hh * 64 + qt * 8 + 8, :],
                        oa[:, qq * DH:(qq + 1) * DH],
                    )

            for ph in range(NH // 2):
                ets = [etp.tile([P, 2048], name=f"et", tag="et", dtype=BF16)
                       for _ in range(QT)]
                last_pair = ph == NH // 2 - 1
                for half in range(2):
                    for kt in range(QT):
                        with tc.high_priority(offset=HIPRI):
                            sps = psS.tile([P, 1024], F32, name="sps", tag="pss")
                            for hh in range(2):
                                bp = hh * DH
                                nc.tensor.matmul(
                                    sps[:, hh * 512:(hh + 1) * 512],
                                    k_sb[ph][bp:bp + DH, kt * P:(kt + 1) * P],
                                    q_sb[ph][bp:bp + DH, half * 512:(half + 1) * 512],
                                    start=True,
                                    stop=True,
                                )
                            nc.scalar.activation(
                                ets[kt][:, half * 1024:(half + 1) * 1024], sps[:],
                                EXP, scale=0.125,
                            )
                        if ph == 0 and half == 0:
                            vt_proj(kt)  # before first reader (AV below)
                        # next-pair projections go through the unit loop under
                        # high priority: they gate the next pair's exp chain
                        if ph + 1 < NH // 2:
                            np1 = ph + 1
                            u = half * 8 + kt
                            if u == 1:
                                with tc.high_priority(offset=HIPRI):
                                    qk_proj_half(np1, q_sb[np1], wq_sb, x_sb, CT, bq_sb, 0)
                            elif u == 3:
                                with tc.high_priority(offset=HIPRI):
                                    qk_proj_half(np1, k_sb[np1], wk_sb, y_sb, CCT, bk_sb, 0)
                            elif u == 9:
                                with tc.high_priority(offset=HIPRI):
                                    qk_proj_half(np1, q_sb[np1], wq_sb, x_sb, CT, bq_sb, 1)
                            elif u == 11:
                                with tc.high_priority(offset=HIPRI):
                                    qk_proj_half(np1, k_sb[np1], wk_sb, y_sb, CCT, bk_sb, 1)
                    # bank `half` only needs the ETs of this half: emit right
                    # after the half's last unit so it overlaps the other half
                    emit_av_bank(ph, 0, ets, half, quirk_spread=last_pair and half == 1)
                    emit_av_bank(ph, 1, ets, half, quirk_spread=last_pair and half == 1)

            # ---- output projection ----
            # split groups across both psum pools so more groups can pre-run
            # their kt=0..2 members before the last pair's quirk lands
            out3 = out.rearrange("(t p) n -> p t n", p=P)
            for ct in range(CT):
                for half in range(2):
                    pool, tag = (psP, "psp") if (ct % 2 == 0) else (psS, "pss")
                    ps = pool.tile([P, 512], F32, name="ps", tag=tag)
                    for kt in range(CT):
                        nc.tensor.matmul(
                            ps[:],
                            wo_sb[kt][:, ct * P:(ct + 1) * P],
                            qk_sb[kt][:, half * 512:(half + 1) * 512],
                            start=(kt == 0),
                            stop=(kt == CT - 1),
                        )
                    ev = evp.tile([P, 512], BF16, name="ev", tag="ev")
                    nc.vector.tensor_scalar_add(ev[:], ps[:], bo_sb[:, ct:ct + 1])
                    dmae[(ct * 2 + half) % 2].dma_start(
                        out3[:, ct, half * 512:(half + 1) * 512], ev[:]
                    )

    nc.compile()
    return nc


def kernel(**inputs) -> np.ndarray:
    bf = ml_dtypes.bfloat16
    x = np.ascontiguousarray(np.asarray(inputs["x"], dtype=np.float32).astype(bf))
    y = np.ascontiguousarray(np.asarray(inputs["y"], dtype=np.float32).astype(bf))
    wqT = np.ascontiguousarray(np.asarray(inputs["w_q"], dtype=np.float32).T.astype(bf))
    wkT = np.ascontiguousarray(np.asarray(inputs["w_k"], dtype=np.float32).T.astype(bf))
    wvT = np.ascontiguousarray(np.asarray(inputs["w_v"], dtype=np.float32).T.astype(bf))
    woT = np.ascontiguousarray(np.asarray(inputs["w_o"], dtype=np.float32).T.astype(bf))
    bq = np.ascontiguousarray(np.asarray(inputs["b_q"], dtype=np.float32))
    bk = np.ascontiguousarray(np.asarray(inputs["b_k"], dtype=np.float32))
    bv = np.ascontiguousarray(np.asarray(inputs["b_v"], dtype=np.float32).astype(bf))
    bo = np.ascontiguousarray(np.asarray(inputs["b_o"], dtype=np.float32))

    if "nc" not in _CACHE:
        _CACHE["nc"] = _build_nc()
    nc = _CACHE["nc"]

    in_maps = []
    for b in range(B):
        in_maps.append({
            "x": np.ascontiguousarray(x[b].reshape(C, N)),
            "y": np.ascontiguousarray(y[b].reshape(CC, N)),
            "wqT": wqT, "wkT": wkT, "wvT": wvT, "woT": woT,
            "bq": bq, "bk": bk, "bv": bv, "bo": bo,
        })
    res = run_bass_kernel_spmd(nc, in_maps, core_ids=list(range(B)))
    return np.stack([
        np.asarray(res.results[b]["out"]).astype(np.float32).reshape(C, HW, HW)
        for b in range(B)
    ])


# revision 5
# speedup vs baseline: 1.0478x; 1.0478x over previous
"""CrossAttention2D Trainium2 kernel (v4).

Sharding: data-parallel over batch. B=8 -> one batch element per NeuronCore,
no collectives. Weights replicated; host pre-transposes and casts to bf16.

Per-core math (C=512, Ccross=768, N=1024, 8 heads x 64):
  Q = Wq @ x_b          [C, N]   bf16
  K = Wk @ y_b          [C, N]   bf16
  VTa = [(Wv @ y_b).T | 1]       [N, 8*(64+1)] bf16 (ones col per head)
  per head pair ph (heads at PE rows 0/64, row-tiled scores):
    S[k, q] = K_h^T Q_h          psum [128, 1024] per (half, kt), ping-pong
    ET = exp(S/8)                ACT -> SBUF bf16, resident for whole pair
    O_aug[q, 0:65] = ET_tile^T @ VTa_h   (ET stationary, 65-col streams,
                                          kt-inner per cell; col 64 = denom)
    O = O_aug[:, :64] / O_aug[:, 64]     (DVE reciprocal + scale) -> bf16
  quirk: out_flat[h*64 + q//16, 64*(q%16) + d] = O_h[q, d]  (DMA shuffle)
  out = Wo @ quirk + bo          [C, N] -> bf16 out, host casts to f32

v4 scheduling (from v2/v3 ntff traces):
  - separate PSUM pools for next-pair Q/K proj (psQK) vs VT proj (psVT):
    in v2/v3 they shared one 2-slot pool, so the exp-critical Q/K proj
    serialized behind low-priority VT work -> 5-14us ACT stall per pair
  - exp-critical chain (Q/K proj, scores, exp) under tc.high_priority
  - consolidated 3D input DMAs (one instr per tensor slice group, ~600ns
    issue each) ordered so exp#1 only waits on ~1MB: wq-ct0, x-h0,
    wk-ct0, y-strip0; K proj for pair 0 follows the slices
  - AV emitted per (pair, bank): bank0 only needs half0 ETs
  - last pair's quirk + output DMAs alternate sync/scalar (ACT idle)
  - out-proj groups ct0/ct1 use psQK/psVT (free during pairs 1-3 -> they
    pre-run), ct2/ct3 use psS (free after the last exp)
"""

import numpy as np
import ml_dtypes

import concourse.bass as bass
import concourse.mybir as mybir
import concourse.tile as tile
from concourse import bacc
from concourse.bass_utils import run_bass_kernel_spmd

P = 128
C = 512          # d_embed
CC = 768         # d_cross
N = 1024         # H*W = 32*32
NH = 8
DH = 64
CT = C // P      # 4
CCT = CC // P    # 6
QT = N // P      # 8
HW = 32
B = 8
F32 = mybir.dt.float32
BF16 = mybir.dt.bfloat16
HIPRI = 1_000_000

_CACHE = {}


def _build_nc():
    nc = bacc.Bacc("TRN2", target_bir_lowering=False, debug=False, num_devices=B)

    x = nc.dram_tensor("x", [C, N], BF16, kind="ExternalInput")
    y = nc.dram_tensor("y", [CC, N], BF16, kind="ExternalInput")
    wqT = nc.dram_tensor("wqT", [C, C], BF16, kind="ExternalInput")
    wkT = nc.dram_tensor("wkT", [CC, C], BF16, kind="ExternalInput")
    wvT = nc.dram_tensor("wvT", [CC, C], BF16, kind="ExternalInput")
    woT = nc.dram_tensor("woT", [C, C], BF16, kind="ExternalInput")
    bq = nc.dram_tensor("bq", [C], F32, kind="ExternalInput")
    bk = nc.dram_tensor("bk", [C], F32, kind="ExternalInput")
    bv = nc.dram_tensor("bv", [C], BF16, kind="ExternalInput")
    bo = nc.dram_tensor("bo", [C], F32, kind="ExternalInput")
    out = nc.dram_tensor("out", [C, N], BF16, kind="ExternalOutput")

    EXP = mybir.ActivationFunctionType.Exp

    with tile.TileContext(nc) as tc:
        with (
            tc.tile_pool(name="const", bufs=1) as constp,
            tc.tile_pool(name="big", bufs=1) as bigp,
            tc.tile_pool(name="et", bufs=16) as etp,
            tc.tile_pool(name="oa", bufs=3) as oap,
            tc.tile_pool(name="ev", bufs=3) as evp,
            tc.tile_pool(name="rcp", bufs=4) as rcpp,
            tc.tile_pool(name="psS", bufs=2, space="PSUM") as psS,
            tc.tile_pool(name="psAV", bufs=2, space="PSUM") as psAV,
            tc.tile_pool(name="psQK", bufs=1, space="PSUM") as psQK,
            tc.tile_pool(name="psVT", bufs=1, space="PSUM") as psVT,
        ):
            # ---- constants ----
            ones_r = constp.tile([1, P], BF16, name="ones_r", tag="ones_r")
            nc.vector.memset(ones_r[:], 1.0)
            # preload the exp table set early so the ~2.7us ACT_TABLE_LOAD
            # overlaps the input DMA phase instead of the first real exp
            dmy = constp.tile([P, 1], F32, name="dmy", tag="dmy")
            nc.vector.memset(dmy[:], 0.0)
            dmy2 = constp.tile([P, 1], F32, name="dmy2", tag="dmy2")
            nc.scalar.activation(dmy2[:], dmy[:], EXP)

            bq_sb = constp.tile([P, CT], F32, name="bq", tag="bq")
            bk_sb = constp.tile([P, CT], F32, name="bk", tag="bk")
            bo_sb = constp.tile([P, CT], F32, name="bo", tag="bo")
            bv_sb = constp.tile([1, C], BF16, name="bv", tag="bv")

            # ---- consolidated input tiles (3D views) ----
            x3 = x.rearrange("(t p) n -> p t n", p=P)
            y3 = y.rearrange("(t p) n -> p t n", p=P)
            wq3 = wqT.rearrange("(t p) m -> p t m", p=P)
            wk3 = wkT.rearrange("(t p) m -> p t m", p=P)
            wv3 = wvT.rearrange("(t p) m -> p t m", p=P)
            wo3 = woT.rearrange("(t p) m -> p t m", p=P)

            xb = bigp.tile([P, CT, N], BF16, name="xb", tag="xb")
            yb = bigp.tile([P, CCT, N], BF16, name="yb", tag="yb")
            wqb = bigp.tile([P, CT, C], BF16, name="wqb", tag="wqb")
            wkb = bigp.tile([P, CCT, C], BF16, name="wkb", tag="wkb")
            wvb = bigp.tile([P, CCT, C], BF16, name="wvb", tag="wvb")
            wob = bigp.tile([P, CT, C], BF16, name="wob", tag="wob")

            # exp#1 critical wave (~1MB): biases for the first adds, then
            # wq ct0 / x half0 / wk ct0 / y strip0, split across queues
            nc.scalar.dma_start(bq_sb[:], bq.rearrange("(o p) -> p o", p=P))
            nc.scalar.dma_start(bk_sb[:], bk.rearrange("(o p) -> p o", p=P))
            nc.sync.dma_start(wqb[:, :, 0:P], wq3[:, :, 0:P])
            nc.scalar.dma_start(xb[:, :, 0:512], x3[:, :, 0:512])
            nc.sync.dma_start(wkb[:, :, 0:P], wk3[:, :, 0:P])
            nc.scalar.dma_start(yb[:, :, 0:P], y3[:, :, 0:P])
            # second wave (all on sync so the scalar queue stays clear for
            # exps): rest of y (K cols 128-1023), x half1, remaining weights
            nc.sync.dma_start(yb[:, :, P:512], y3[:, :, P:512])
            nc.sync.dma_start(yb[:, :, 512:N], y3[:, :, 512:N])
            nc.sync.dma_start(xb[:, :, 512:N], x3[:, :, 512:N])
            nc.sync.dma_start(wqb[:, :, P:C], wq3[:, :, P:C])
            nc.sync.dma_start(wkb[:, :, P:C], wk3[:, :, P:C])
            nc.sync.dma_start(wvb[:], wv3[:])
            nc.sync.dma_start(wob[:], wo3[:])
            nc.sync.dma_start(bo_sb[:], bo.rearrange("(o p) -> p o", p=P))
            nc.sync.dma_start(bv_sb[:], bv[None, :])

            q_sb = [bigp.tile([P, N], BF16, name=f"q{t}", tag=f"q{t}") for t in range(CT)]
            k_sb = [bigp.tile([P, N], BF16, name=f"k{t}", tag=f"k{t}") for t in range(CT)]
            # VTa buffer: per n-tile, cols laid out [h][65] with col h*65+64 == 1.0
            vt_sb = [bigp.tile([P, NH * (DH + 1)], BF16, name=f"vt{t}", tag=f"vt{t}")
                     for t in range(QT)]
            for t in range(QT):
                nc.gpsimd.memset(vt_sb[t][:], 1.0)

            # ---- projection helpers ----
            def qk_proj_cols(ct, dst, wb, srcb, nkt, bias_sb, c0, c1):
                """dst[:, c0:c1] = (W @ src)[ct*P:(ct+1)*P, c0:c1] + bias."""
                ps = psQK.tile([P, 512], F32, name="ps", tag="psqk")
                for kt in range(nkt):
                    nc.tensor.matmul(
                        ps[:, 0:c1 - c0],
                        wb[:, kt, ct * P:(ct + 1) * P],
                        srcb[:, kt, c0:c1],
                        start=(kt == 0),
                        stop=(kt == nkt - 1),
                    )
                nc.vector.tensor_scalar_add(
                    dst[:, c0:c1], ps[:, 0:c1 - c0], bias_sb[:, ct:ct + 1]
                )

            # ---- VT projection: VT[n, c] = sum_k y[k, n] * wvT[k, c]  (+ bias row)
            def vt_proj(nt):
                ps = psVT.tile([P, 512], F32, name="ps", tag="psvt")
                for kt in range(CCT):
                    nc.tensor.matmul(
                        ps[:],
                        yb[:, kt, nt * P:(nt + 1) * P],
                        wvb[:, kt, :],
                        start=(kt == 0),
                        stop=False,
                    )
                nc.tensor.matmul(ps[:], ones_r[:], bv_sb[:], start=False, stop=True)
                # scatter into [h][0:64] slots (col h*65+64 stays 1.0)
                nc.vector.tensor_copy(
                    out=vt_sb[nt].rearrange("p (h e) -> p h e", e=DH + 1)[:, :, 0:DH],
                    in_=ps.rearrange("p (h d) -> p h d", d=DH),
                )

            # pair-0 prologue: K in three column chunks chasing the y DMAs,
            # so exp#1 only waits on y strip0
            with tc.high_priority(offset=HIPRI):
                qk_proj_cols(0, q_sb[0], wqb, xb, CT, bq_sb, 0, 512)
                qk_proj_cols(0, k_sb[0], wkb, yb, CCT, bk_sb, 0, P)
                qk_proj_cols(0, k_sb[0], wkb, yb, CCT, bk_sb, P, 512)
                qk_proj_cols(0, q_sb[0], wqb, xb, CT, bq_sb, 512, N)
                qk_proj_cols(0, k_sb[0], wkb, yb, CCT, bk_sb, 512, N)

            # ---- attention ----
            qk_sb = [bigp.tile([P, N], BF16, name=f"qk{t}", tag=f"qk{t}")
                     for t in range(CT)]

            def ecol(hh, qt):
                return (qt // 4) * 1024 + hh * 512 + (qt % 4) * P

            def emit_av_bank(ph, hh, ets, bank, quirk_spread):
                """AV for one (head, 4-qt bank). Cell accumulation groups within
                one PSUM bank must be sequential (start=True clears has_written
                for the whole bank), so cells run kt-inner back-to-back."""
                h = 2 * ph + hh
                oa = oap.tile([P, 256], BF16, name="oa", tag="oa")
                av = psAV.tile([P, 512], F32, name="av", tag="av")
                for qq in range(4):
                    qt = bank * 4 + qq
                    for kt in range(QT):
                        nc.tensor.matmul(
                            av[:, qq * P:qq * P + DH + 1],
                            ets[kt][:, ecol(hh, qt):ecol(hh, qt) + P],
                            vt_sb[kt][:, h * (DH + 1):(h + 1) * (DH + 1)],
                            start=(kt == 0),
                            stop=(kt == QT - 1),
                        )
                # batched normalize: one reciprocal for the bank's 4
                # denominators, then per-cell scale + shuffle
                rcp = rcpp.tile([P, 4], F32, name="rcp", tag="rcp")
                nc.vector.reciprocal(
                    rcp[:], av.rearrange("p (q c) -> p q c", c=P)[:, :, DH]
                )
                for qq in range(4):
                    qt = bank * 4 + qq
                    nc.vector.tensor_scalar_mul(
                        oa[:, qq * DH:(qq + 1) * DH],
                        av[:, qq * P:qq * P + DH], rcp[:, qq:qq + 1],
                    )
                    # quirk shuffle:
                    # qk[ph][hh*64 + qt*8 + p//16, 64*(p%16)+d] = O_h[qt*128+p, d]
                    eng = nc.scalar if (quirk_spread and qq % 2 == 1) else nc.sync
                    eng.dma_start(
                        qk_sb[ph][hh * 64 + qt * 8: hh * 64 + qt * 8 + 8, :],
                        oa[:, qq * DH:(qq + 1) * DH],
                    )

            for ph in range(NH // 2):
                ets = [etp.tile([P, 2048], name="et", tag="et", dtype=BF16)
                       for _ in range(QT)]
                last_pair = ph == NH // 2 - 1
                for half in range(2):
                    for kt in range(QT):
                        with tc.high_priority(offset=HIPRI):
                            sps = psS.tile([P, 1024], F32, name="sps", tag="pss")
                            for hh in range(2):
                                bp = hh * DH
                                nc.tensor.matmul(
                                    sps[:, hh * 512:(hh + 1) * 512],
                                    k_sb[ph][bp:bp + DH, kt * P:(kt + 1) * P],
                                    q_sb[ph][bp:bp + DH, half * 512:(half + 1) * 512],
                                    start=True,
                                    stop=True,
                                )
                            nc.scalar.activation(
                                ets[kt][:, half * 1024:(half + 1) * 1024], sps[:],
                                EXP, scale=0.125,
                            )
                        if ph == 0 and half == 0:
                            vt_proj(kt)  # before first reader (AV below)
                        # next-pair projections gate the next pair's exp chain
                        if ph + 1 < NH // 2:
                            np1 = ph + 1
                            u = half * 8 + kt
                            if u == 1:
                                with tc.high_priority(offset=HIPRI):
                                    qk_proj_cols(np1, q_sb[np1], wqb, xb, CT, bq_sb, 0, 512)
                            elif u == 3:
                                with tc.high_priority(offset=HIPRI):
                                    qk_proj_cols(np1, k_sb[np1], wkb, yb, CCT, bk_sb, 0, 512)
                            elif u == 9:
                                with tc.high_priority(offset=HIPRI):
                                    qk_proj_cols(np1, q_sb[np1], wqb, xb, CT, bq_sb, 512, N)
                            elif u == 11:
                                with tc.high_priority(offset=HIPRI):
                                    qk_proj_cols(np1, k_sb[np1], wkb, yb, CCT, bk_sb, 512, N)
                    # bank `half` only needs the ETs of this half: emit right
                    # after the half's last unit so it overlaps the other half
                    emit_av_bank(ph, 0, ets, half, quirk_spread=last_pair and half == 1)
                    emit_av_bank(ph, 1, ets, half, quirk_spread=last_pair and half == 1)

            # ---- output projection ----
            # ct0/ct1 groups take psQK/psVT (free during pairs 1-3, so their
            # kt=0..2 members pre-run); ct2/ct3 take psS (free after last exp)
            out3 = out.rearrange("(t p) n -> p t n", p=P)
            for ct in range(CT):
                for half in range(2):
                    pool, tag = [(psQK, "psqk"), (psVT, "psvt"),
                                 (psS, "pss"), (psS, "pss")][ct]
                    ps = pool.tile([P, 512], F32, name="ps", tag=tag)
                    for kt in range(CT):
                        nc.tensor.matmul(
                            ps[:, 0:512],
                            wob[:, kt, ct * P:(ct + 1) * P],
                            qk_sb[kt][:, half * 512:(half + 1) * 512],
                            start=(kt == 0),
                            stop=(kt == CT - 1),
                        )
                    ev = evp.tile([P, 512], BF16, name="ev", tag="ev")
                    nc.vector.tensor_scalar_add(ev[:], ps[:, 0:512], bo_sb[:, ct:ct + 1])
                    eng = nc.scalar if half == 1 else nc.sync
                    eng.dma_start(out3[:, ct, half * 512:(half + 1) * 512], ev[:])

    nc.compile()
    return nc


def kernel(**inputs) -> np.ndarray:
    bf = ml_dtypes.bfloat16
    x = np.ascontiguousarray(np.asarray(inputs["x"], dtype=np.float32).astype(bf))
    y = np.ascontiguousarray(np.asarray(inputs["y"], dtype=np.float32).astype(bf))
    wqT = np.ascontiguousarray(np.asarray(inputs["w_q"], dtype=np.float32).T.astype(bf))
    wkT = np.ascontiguousarray(np.asarray(inputs["w_k"], dtype=np.float32).T.astype(bf))
    wvT = np.ascontiguousarray(np.asarray(inputs["w_v"], dtype=np.float32).T.astype(bf))
    woT = np.ascontiguousarray(np.asarray(inputs["w_o"], dtype=np.float32).T.astype(bf))
    bq = np.ascontiguousarray(np.asarray(inputs["b_q"], dtype=np.float32))
    bk = np.ascontiguousarray(np.asarray(inputs["b_k"], dtype=np.float32))
    bv = np.ascontiguousarray(np.asarray(inputs["b_v"], dtype=np.float32).astype(bf))
    bo = np.ascontiguousarray(np.asarray(inputs["b_o"], dtype=np.float32))

    if "nc" not in _CACHE:
        _CACHE["nc"] = _build_nc()
    nc = _CACHE["nc"]

    in_maps = []
    for b in range(B):
        in_maps.append({
            "x": np.ascontiguousarray(x[b].reshape(C, N)),
            "y": np.ascontiguousarray(y[b].reshape(CC, N)),
            "wqT": wqT, "wkT": wkT, "wvT": wvT, "woT": woT,
            "bq": bq, "bk": bk, "bv": bv, "bo": bo,
        })
    res = run_bass_kernel_spmd(nc, in_maps, core_ids=list(range(B)))
    return np.stack([
        np.asarray(res.results[b]["out"]).astype(np.float32).reshape(C, HW, HW)
        for b in range(B)
    ])


# revision 12
# speedup vs baseline: 1.1043x; 1.0539x over previous
"""CrossAttention2D Trainium2 kernel (v4).

Sharding: data-parallel over batch. B=8 -> one batch element per NeuronCore,
no collectives. Weights replicated; host pre-transposes and casts to bf16.

Per-core math (C=512, Ccross=768, N=1024, 8 heads x 64):
  Q = Wq @ x_b          [C, N]   bf16
  K = Wk @ y_b          [C, N]   bf16
  VTa = [(Wv @ y_b).T | 1]       [N, 8*(64+1)] bf16 (ones col per head)
  per head pair ph (heads at PE rows 0/64, row-tiled scores):
    S[k, q] = K_h^T Q_h          psum [128, 1024] per (half, kt), ping-pong
    ET = exp(S/8)                ACT -> SBUF bf16, resident for whole pair
    O_aug[q, 0:65] = ET_tile^T @ VTa_h   (ET stationary, 65-col streams,
                                          kt-inner per cell; col 64 = denom)
    O = O_aug[:, :64] / O_aug[:, 64]     (DVE reciprocal + scale) -> bf16
  quirk: out_flat[h*64 + q//16, 64*(q%16) + d] = O_h[q, d]  (DMA shuffle)
  out = Wo @ quirk + bo          [C, N] -> bf16 out, host casts to f32

v4 scheduling (from v2/v3 ntff traces):
  - separate PSUM pools for next-pair Q/K proj (psQK) vs VT proj (psVT):
    in v2/v3 they shared one 2-slot pool, so the exp-critical Q/K proj
    serialized behind low-priority VT work -> 5-14us ACT stall per pair
  - exp-critical chain (Q/K proj, scores, exp) under tc.high_priority
  - consolidated 3D input DMAs (one instr per tensor slice group, ~600ns
    issue each) ordered so exp#1 only waits on ~1MB: wq-ct0, x-h0,
    wk-ct0, y-strip0; K proj for pair 0 follows the slices
  - AV emitted per (pair, bank): bank0 only needs half0 ETs
  - last pair's quirk + output DMAs alternate sync/scalar (ACT idle)
  - out-proj groups ct0/ct1 use psQK/psVT (free during pairs 1-3 -> they
    pre-run), ct2/ct3 use psS (free after the last exp)
"""

import numpy as np
import ml_dtypes

import concourse.bass as bass
import concourse.mybir as mybir
import concourse.tile as tile
from concourse import bacc
from concourse.bass_utils import run_bass_kernel_spmd

P = 128
C = 512          # d_embed
CC = 768         # d_cross
N = 1024         # H*W = 32*32
NH = 8
DH = 64
CT = C // P      # 4
CCT = CC // P    # 6
QT = N // P      # 8
HW = 32
B = 8
F32 = mybir.dt.float32
BF16 = mybir.dt.bfloat16
HIPRI = 1_000_000

_CACHE = {}


def _build_nc():
    nc = bacc.Bacc("TRN2", target_bir_lowering=False, debug=False, num_devices=B)

    x = nc.dram_tensor("x", [C, N], BF16, kind="ExternalInput")
    y = nc.dram_tensor("y", [CC, N], BF16, kind="ExternalInput")
    wqT = nc.dram_tensor("wqT", [C, C], BF16, kind="ExternalInput")
    wkT = nc.dram_tensor("wkT", [CC, C], BF16, kind="ExternalInput")
    wvT = nc.dram_tensor("wvT", [CC, C], BF16, kind="ExternalInput")
    woT = nc.dram_tensor("woT", [C, C], BF16, kind="ExternalInput")
    bq = nc.dram_tensor("bq", [C], F32, kind="ExternalInput")
    bk = nc.dram_tensor("bk", [C], F32, kind="ExternalInput")
    bv = nc.dram_tensor("bv", [C], BF16, kind="ExternalInput")
    bo = nc.dram_tensor("bo", [C], F32, kind="ExternalInput")
    out = nc.dram_tensor("out", [C, N], BF16, kind="ExternalOutput")

    EXP = mybir.ActivationFunctionType.Exp

    with tile.TileContext(nc) as tc:
        with (
            tc.tile_pool(name="const", bufs=1) as constp,
            tc.tile_pool(name="big", bufs=1) as bigp,
            tc.tile_pool(name="et", bufs=24) as etp,
            tc.tile_pool(name="oa", bufs=4) as oap,
            tc.tile_pool(name="ev", bufs=3) as evp,
            tc.tile_pool(name="rcp", bufs=4) as rcpp,
            tc.tile_pool(name="psS", bufs=2, space="PSUM") as psS,
            tc.tile_pool(name="psAV", bufs=2, space="PSUM") as psAV,
            tc.tile_pool(name="psQK", bufs=1, space="PSUM") as psQK,
            tc.tile_pool(name="psVT", bufs=1, space="PSUM") as psVT,
        ):
            # ---- constants ----
            ones_r = constp.tile([1, P], BF16, name="ones_r", tag="ones_r")
            nc.vector.memset(ones_r[:], 1.0)
            # preload the exp table set early so the ~2.7us ACT_TABLE_LOAD
            # overlaps the input DMA phase instead of the first real exp
            dmy = constp.tile([P, 1], F32, name="dmy", tag="dmy")
            nc.vector.memset(dmy[:], 0.0)
            dmy2 = constp.tile([P, 1], F32, name="dmy2", tag="dmy2")
            nc.scalar.activation(dmy2[:], dmy[:], EXP)

            bq_sb = constp.tile([P, CT], F32, name="bq", tag="bq")
            bk_sb = constp.tile([P, CT], F32, name="bk", tag="bk")
            bo_sb = constp.tile([P, CT], F32, name="bo", tag="bo")
            bv_sb = constp.tile([1, C], BF16, name="bv", tag="bv")

            # ---- consolidated input tiles (3D views) ----
            x3 = x.rearrange("(t p) n -> p t n", p=P)
            y3 = y.rearrange("(t p) n -> p t n", p=P)
            wq3 = wqT.rearrange("(t p) m -> p t m", p=P)
            wk3 = wkT.rearrange("(t p) m -> p t m", p=P)
            wv3 = wvT.rearrange("(t p) m -> p t m", p=P)
            wo3 = woT.rearrange("(t p) m -> p t m", p=P)

            xb = bigp.tile([P, CT, N], BF16, name="xb", tag="xb")
            yb = bigp.tile([P, CCT, N], BF16, name="yb", tag="yb")
            wqb = bigp.tile([P, CT, C], BF16, name="wqb", tag="wqb")
            wkb = bigp.tile([P, CCT, C], BF16, name="wkb", tag="wkb")
            wvb = bigp.tile([P, CCT, C], BF16, name="wvb", tag="wvb")
            wob = bigp.tile([P, CT, C], BF16, name="wob", tag="wob")

            # all input DMAs on ONE queue (sync) in strict priority order:
            # the DGE issue order is also the HBM bandwidth allocation order,
            # so eager second-wave DMAs must not race the exp#1-critical wave
            # (v4 lost ~5us to x-h0 sharing bandwidth with later loads)
            nc.scalar.dma_start(bq_sb[:], bq.rearrange("(o p) -> p o", p=P))
            nc.scalar.dma_start(bk_sb[:], bk.rearrange("(o p) -> p o", p=P))
            nc.sync.dma_start(wqb[:, :, 0:P], wq3[:, :, 0:P])
            nc.sync.dma_start(xb[:, :, 0:512], x3[:, :, 0:512])
            nc.sync.dma_start(wkb[:, :, 0:P], wk3[:, :, 0:P])
            nc.sync.dma_start(yb[:, :, 0:P], y3[:, :, 0:P])
            # second wave, in order of first use
            nc.sync.dma_start(yb[:, :, P:512], y3[:, :, P:512])
            nc.sync.dma_start(yb[:, :, 512:N], y3[:, :, 512:N])
            nc.sync.dma_start(xb[:, :, 512:N], x3[:, :, 512:N])
            nc.sync.dma_start(wqb[:, :, P:C], wq3[:, :, P:C])
            nc.sync.dma_start(wkb[:, :, P:C], wk3[:, :, P:C])
            nc.sync.dma_start(wvb[:], wv3[:])
            nc.sync.dma_start(wob[:], wo3[:])
            nc.sync.dma_start(bo_sb[:], bo.rearrange("(o p) -> p o", p=P))
            nc.sync.dma_start(bv_sb[:], bv[None, :])

            q_sb = [bigp.tile([P, N], BF16, name=f"q{t}", tag=f"q{t}") for t in range(CT)]
            k_sb = [bigp.tile([P, N], BF16, name=f"k{t}", tag=f"k{t}") for t in range(CT)]
            # VTa buffer: per n-tile, cols laid out [h][65] with col h*65+64 == 1.0
            vt_sb = [bigp.tile([P, NH * (DH + 1)], BF16, name=f"vt{t}", tag=f"vt{t}")
                     for t in range(QT)]
            for t in range(QT):
                nc.gpsimd.memset(vt_sb[t][:], 1.0)

            # PE warm-up: dummy matmuls while the input DMAs stream, so the
            # HAM clock gate is at 8/8 before the first projection and exp#1
            # isn't paying cold-clock prices (~3.4us of sustained PE activity
            # flips the gate)
            wup = psVT.tile([P, 512], F32, name="wup", tag="psvt")
            for i in range(18):
                nc.tensor.matmul(
                    wup[:], vt_sb[6][:, 0:P], vt_sb[7][:, 0:512],
                    start=True, stop=True,
                )

            # ---- projection helpers ----
            def qk_proj_cols(ct, dst, wb, srcb, nkt, bias_sb, c0, c1):
                """dst[:, c0:c1] = (W @ src)[ct*P:(ct+1)*P, c0:c1] + bias."""
                ps = psQK.tile([P, 512], F32, name="ps", tag="psqk")
                for kt in range(nkt):
                    nc.tensor.matmul(
                        ps[:, 0:c1 - c0],
                        wb[:, kt, ct * P:(ct + 1) * P],
                        srcb[:, kt, c0:c1],
                        start=(kt == 0),
                        stop=(kt == nkt - 1),
                    )
                nc.vector.tensor_scalar_add(
                    dst[:, c0:c1], ps[:, 0:c1 - c0], bias_sb[:, ct:ct + 1]
                )

            # ---- VT projection: VT[n, c] = sum_k y[k, n] * wvT[k, c]  (+ bias row)
            def vt_proj(nt):
                ps = psVT.tile([P, 512], F32, name="ps", tag="psvt")
                for kt in range(CCT):
                    nc.tensor.matmul(
                        ps[:],
                        yb[:, kt, nt * P:(nt + 1) * P],
                        wvb[:, kt, :],
                        start=(kt == 0),
                        stop=False,
                    )
                nc.tensor.matmul(ps[:], ones_r[:], bv_sb[:], start=False, stop=True)
                # scatter into [h][0:64] slots (col h*65+64 stays 1.0)
                nc.vector.tensor_copy(
                    out=vt_sb[nt].rearrange("p (h e) -> p h e", e=DH + 1)[:, :, 0:DH],
                    in_=ps.rearrange("p (h d) -> p h d", d=DH),
                )

            # pair-0 prologue: K in three column chunks chasing the y DMAs,
            # so exp#1 only waits on y strip0
            with tc.high_priority(offset=HIPRI):
                qk_proj_cols(0, q_sb[0], wqb, xb, CT, bq_sb, 0, 512)
                qk_proj_cols(0, k_sb[0], wkb, yb, CCT, bk_sb, 0, P)
                qk_proj_cols(0, k_sb[0], wkb, yb, CCT, bk_sb, P, 512)
                qk_proj_cols(0, q_sb[0], wqb, xb, CT, bq_sb, 512, N)
                qk_proj_cols(0, k_sb[0], wkb, yb, CCT, bk_sb, 512, N)

            # ---- attention ----
            qk_sb = [bigp.tile([P, N], BF16, name=f"qk{t}", tag=f"qk{t}")
                     for t in range(CT)]

            def ecol(hh, qt):
                return (qt // 4) * 1024 + hh * 512 + (qt % 4) * P

            def emit_av_bank(ph, hh, ets, bank, quirk_spread):
                """AV for one (head, 4-qt bank). Cell accumulation groups within
                one PSUM bank must be sequential (start=True clears has_written
                for the whole bank), so cells run kt-inner back-to-back."""
                h = 2 * ph + hh
                oa = oap.tile([P, 256], BF16, name="oa", tag="oa")
                av = psAV.tile([P, 512], F32, name="av", tag="av")
                for qq in range(4):
                    qt = bank * 4 + qq
                    for kt in range(QT):
                        nc.tensor.matmul(
                            av[:, qq * P:qq * P + DH + 1],
                            ets[kt][:, ecol(hh, qt):ecol(hh, qt) + P],
                            vt_sb[kt][:, h * (DH + 1):(h + 1) * (DH + 1)],
                            start=(kt == 0),
                            stop=(kt == QT - 1),
                        )
                # batched normalize: one reciprocal for the bank's 4
                # denominators, one broadcast multiply for all 4 cells
                rcp = rcpp.tile([P, 4], F32, name="rcp", tag="rcp")
                nc.vector.reciprocal(
                    rcp[:], av.rearrange("p (q c) -> p q c", c=P)[:, :, DH]
                )
                for qq in range(4):
                    nc.vector.tensor_scalar_mul(
                        oa[:, qq * DH:(qq + 1) * DH],
                        av[:, qq * P:qq * P + DH], rcp[:, qq:qq + 1],
                    )
                # quirk shuffle:
                # qk[ph][hh*64 + qt*8 + p//16, 64*(p%16)+d] = O_h[qt*128+p, d]
                for qq in range(4):
                    qt = bank * 4 + qq
                    eng = nc.scalar if (quirk_spread and qq % 2 == 1) else nc.sync
                    eng.dma_start(
                        qk_sb[ph][hh * 64 + qt * 8: hh * 64 + qt * 8 + 8, :],
                        oa[:, qq * DH:(qq + 1) * DH],
                    )

            for ph in range(NH // 2):
                ets = [etp.tile([P, 2048], name="et", tag="et", dtype=BF16)
                       for _ in range(QT)]
                last_pair = ph == NH // 2 - 1
                for half in range(2):
                    for kt in range(QT):
                        with tc.high_priority(offset=HIPRI):
                            sps = psS.tile([P, 1024], F32, name="sps", tag="pss")
                            for hh in range(2):
                                bp = hh * DH
                                nc.tensor.matmul(
                                    sps[:, hh * 512:(hh + 1) * 512],
                                    k_sb[ph][bp:bp + DH, kt * P:(kt + 1) * P],
                                    q_sb[ph][bp:bp + DH, half * 512:(half + 1) * 512],
                                    start=True,
                                    stop=True,
                                )
                            nc.scalar.activation(
                                ets[kt][:, half * 1024:(half + 1) * 1024], sps[:],
                                EXP, scale=0.125,
                            )
                        if ph == 0 and half == 0:
                            vt_proj(kt)  # before first reader (AV below)
                        # next-pair projections gate the next pair's exp chain
                        if ph + 1 < NH // 2:
                            np1 = ph + 1
                            u = half * 8 + kt
                            if u == 1:
                                with tc.high_priority(offset=HIPRI):
                                    qk_proj_cols(np1, q_sb[np1], wqb, xb, CT, bq_sb, 0, 512)
                            elif u == 3:
                                with tc.high_priority(offset=HIPRI):
                                    qk_proj_cols(np1, k_sb[np1], wkb, yb, CCT, bk_sb, 0, 512)
                            elif u == 9:
                                with tc.high_priority(offset=HIPRI):
                                    qk_proj_cols(np1, q_sb[np1], wqb, xb, CT, bq_sb, 512, N)
                            elif u == 11:
                                with tc.high_priority(offset=HIPRI):
                                    qk_proj_cols(np1, k_sb[np1], wkb, yb, CCT, bk_sb, 512, N)
                    # bank `half` only needs the ETs of this half: emit right
                    # after the half's last unit so it overlaps the other half
                    emit_av_bank(ph, 0, ets, half, quirk_spread=last_pair and half == 1)
                    emit_av_bank(ph, 1, ets, half, quirk_spread=last_pair and half == 1)

            # ---- output projection ----
            # ct0/ct1 groups take psQK/psVT (free during pairs 1-3, so their
            # kt=0..2 members pre-run); ct2/ct3 take psS (free after last exp)
            out3 = out.rearrange("(t p) n -> p t n", p=P)
            for ct in range(CT):
                for half in range(2):
                    pool, tag = [(psQK, "psqk"), (psVT, "psvt"),
                                 (psS, "pss"), (psS, "pss")][ct]
                    ps = pool.tile([P, 512], F32, name="ps", tag=tag)
                    for kt in range(CT):
                        nc.tensor.matmul(
                            ps[:, 0:512],
                            wob[:, kt, ct * P:(ct + 1) * P],
                            qk_sb[kt][:, half * 512:(half + 1) * 512],
                            start=(kt == 0),
                            stop=(kt == CT - 1),
                        )
                    ev = evp.tile([P, 512], BF16, name="ev", tag="ev")
                    nc.vector.tensor_scalar_add(ev[:], ps[:, 0:512], bo_sb[:, ct:ct + 1])
                    eng = nc.scalar if half == 1 else nc.sync
                    eng.dma_start(out3[:, ct, half * 512:(half + 1) * 512], ev[:])

    nc.compile()
    return nc


def kernel(**inputs) -> np.ndarray:
    bf = ml_dtypes.bfloat16
    x = np.ascontiguousarray(np.asarray(inputs["x"], dtype=np.float32).astype(bf))
    y = np.ascontiguousarray(np.asarray(inputs["y"], dtype=np.float32).astype(bf))
    wqT = np.ascontiguousarray(np.asarray(inputs["w_q"], dtype=np.float32).T.astype(bf))
    wkT = np.ascontiguousarray(np.asarray(inputs["w_k"], dtype=np.float32).T.astype(bf))
    wvT = np.ascontiguousarray(np.asarray(inputs["w_v"], dtype=np.float32).T.astype(bf))
    woT = np.ascontiguousarray(np.asarray(inputs["w_o"], dtype=np.float32).T.astype(bf))
    bq = np.ascontiguousarray(np.asarray(inputs["b_q"], dtype=np.float32))
    bk = np.ascontiguousarray(np.asarray(inputs["b_k"], dtype=np.float32))
    bv = np.ascontiguousarray(np.asarray(inputs["b_v"], dtype=np.float32).astype(bf))
    bo = np.ascontiguousarray(np.asarray(inputs["b_o"], dtype=np.float32))

    if "nc" not in _CACHE:
        _CACHE["nc"] = _build_nc()
    nc = _CACHE["nc"]

    in_maps = []
    for b in range(B):
        in_maps.append({
            "x": np.ascontiguousarray(x[b].reshape(C, N)),
            "y": np.ascontiguousarray(y[b].reshape(CC, N)),
            "wqT": wqT, "wkT": wkT, "wvT": wvT, "woT": woT,
            "bq": bq, "bk": bk, "bv": bv, "bo": bo,
        })
    res = run_bass_kernel_spmd(nc, in_maps, core_ids=list(range(B)))
    return np.stack([
        np.asarray(res.results[b]["out"]).astype(np.float32).reshape(C, HW, HW)
        for b in range(B)
    ])


# revision 16
# speedup vs baseline: 1.1399x; 1.0323x over previous
"""CrossAttention2D Trainium2 kernel (v4).

Sharding: data-parallel over batch. B=8 -> one batch element per NeuronCore,
no collectives. Weights replicated; host pre-transposes and casts to bf16.

Per-core math (C=512, Ccross=768, N=1024, 8 heads x 64):
  Q = Wq @ x_b          [C, N]   bf16
  K = Wk @ y_b          [C, N]   bf16
  VTa = [(Wv @ y_b).T | 1]       [N, 8*(64+1)] bf16 (ones col per head)
  per head pair ph (heads at PE rows 0/64, row-tiled scores):
    S[k, q] = K_h^T Q_h          psum [128, 1024] per (half, kt), ping-pong
    ET = exp(S/8)                ACT -> SBUF bf16, resident for whole pair
    O_aug[q, 0:65] = ET_tile^T @ VTa_h   (ET stationary, 65-col streams,
                                          kt-inner per cell; col 64 = denom)
    O = O_aug[:, :64] / O_aug[:, 64]     (DVE reciprocal + scale) -> bf16
  quirk: out_flat[h*64 + q//16, 64*(q%16) + d] = O_h[q, d]  (DMA shuffle)
  out = Wo @ quirk + bo          [C, N] -> bf16 out, host casts to f32

v4 scheduling (from v2/v3 ntff traces):
  - separate PSUM pools for next-pair Q/K proj (psQK) vs VT proj (psVT):
    in v2/v3 they shared one 2-slot pool, so the exp-critical Q/K proj
    serialized behind low-priority VT work -> 5-14us ACT stall per pair
  - exp-critical chain (Q/K proj, scores, exp) under tc.high_priority
  - consolidated 3D input DMAs (one instr per tensor slice group, ~600ns
    issue each) ordered so exp#1 only waits on ~1MB: wq-ct0, x-h0,
    wk-ct0, y-strip0; K proj for pair 0 follows the slices
  - AV emitted per (pair, bank): bank0 only needs half0 ETs
  - last pair's quirk + output DMAs alternate sync/scalar (ACT idle)
  - out-proj groups ct0/ct1 use psQK/psVT (free during pairs 1-3 -> they
    pre-run), ct2/ct3 use psS (free after the last exp)
"""

import numpy as np
import ml_dtypes

import concourse.bass as bass
import concourse.mybir as mybir
import concourse.tile as tile
from concourse import bacc
from concourse.bass_utils import run_bass_kernel_spmd

P = 128
C = 512          # d_embed
CC = 768         # d_cross
N = 1024         # H*W = 32*32
NH = 8
DH = 64
CT = C // P      # 4
CCT = CC // P    # 6
QT = N // P      # 8
HW = 32
B = 8
F32 = mybir.dt.float32
BF16 = mybir.dt.bfloat16
HIPRI = 1_000_000

_CACHE = {}


def _build_nc():
    nc = bacc.Bacc("TRN2", target_bir_lowering=False, debug=False, num_devices=B)

    x = nc.dram_tensor("x", [C, N], BF16, kind="ExternalInput")
    y = nc.dram_tensor("y", [CC, N], BF16, kind="ExternalInput")
    wqT = nc.dram_tensor("wqT", [C, C], BF16, kind="ExternalInput")
    wkT = nc.dram_tensor("wkT", [CC, C], BF16, kind="ExternalInput")
    wvT = nc.dram_tensor("wvT", [CC, C], BF16, kind="ExternalInput")
    woT = nc.dram_tensor("woT", [C, C], BF16, kind="ExternalInput")
    bq = nc.dram_tensor("bq", [C], F32, kind="ExternalInput")
    bk = nc.dram_tensor("bk", [C], F32, kind="ExternalInput")
    bv = nc.dram_tensor("bv", [C], BF16, kind="ExternalInput")
    bo = nc.dram_tensor("bo", [C], F32, kind="ExternalInput")
    out = nc.dram_tensor("out", [C, N], BF16, kind="ExternalOutput")

    EXP = mybir.ActivationFunctionType.Exp

    with tile.TileContext(nc) as tc:
        with (
            tc.tile_pool(name="const", bufs=1) as constp,
            tc.tile_pool(name="big", bufs=1) as bigp,
            tc.tile_pool(name="et", bufs=24) as etp,
            tc.tile_pool(name="oa", bufs=4) as oap,
            tc.tile_pool(name="ev", bufs=3) as evp,
            tc.tile_pool(name="rcp", bufs=4) as rcpp,
            tc.tile_pool(name="psS", bufs=2, space="PSUM") as psS,
            tc.tile_pool(name="psAV", bufs=2, space="PSUM") as psAV,
            tc.tile_pool(name="psQK", bufs=1, space="PSUM") as psQK,
            tc.tile_pool(name="psVT", bufs=1, space="PSUM") as psVT,
        ):
            # ---- constants ----
            ones_r = constp.tile([1, P], BF16, name="ones_r", tag="ones_r")
            nc.vector.memset(ones_r[:], 1.0)
            # preload the exp table set early so the ~2.7us ACT_TABLE_LOAD
            # overlaps the input DMA phase instead of the first real exp
            dmy = constp.tile([P, 1], F32, name="dmy", tag="dmy")
            nc.vector.memset(dmy[:], 0.0)
            dmy2 = constp.tile([P, 1], F32, name="dmy2", tag="dmy2")
            nc.scalar.activation(dmy2[:], dmy[:], EXP)

            bq_sb = constp.tile([P, CT], F32, name="bq", tag="bq")
            bk_sb = constp.tile([P, CT], F32, name="bk", tag="bk")
            bo_sb = constp.tile([P, CT], F32, name="bo", tag="bo")
            bv_sb = constp.tile([1, C], BF16, name="bv", tag="bv")

            # ---- consolidated input tiles (3D views) ----
            x3 = x.rearrange("(t p) n -> p t n", p=P)
            y3 = y.rearrange("(t p) n -> p t n", p=P)
            wq3 = wqT.rearrange("(t p) m -> p t m", p=P)
            wk3 = wkT.rearrange("(t p) m -> p t m", p=P)
            wv3 = wvT.rearrange("(t p) m -> p t m", p=P)
            wo3 = woT.rearrange("(t p) m -> p t m", p=P)

            xb = bigp.tile([P, CT, N], BF16, name="xb", tag="xb")
            yb = bigp.tile([P, CCT, N], BF16, name="yb", tag="yb")
            wqb = bigp.tile([P, CT, C], BF16, name="wqb", tag="wqb")
            wkb = bigp.tile([P, CCT, C], BF16, name="wkb", tag="wkb")
            wvb = bigp.tile([P, CCT, C], BF16, name="wvb", tag="wvb")
            wob = bigp.tile([P, CT, C], BF16, name="wob", tag="wob")

            # all input DMAs on ONE queue (sync) in strict priority order:
            # the DGE issue order is also the HBM bandwidth allocation order,
            # so eager second-wave DMAs must not race the exp#1-critical wave
            # (v4 lost ~5us to x-h0 sharing bandwidth with later loads)
            nc.scalar.dma_start(bq_sb[:], bq.rearrange("(o p) -> p o", p=P))
            nc.scalar.dma_start(bk_sb[:], bk.rearrange("(o p) -> p o", p=P))
            nc.sync.dma_start(wqb[:, :, 0:P], wq3[:, :, 0:P])
            nc.sync.dma_start(xb[:, :, 0:512], x3[:, :, 0:512])
            nc.sync.dma_start(wkb[:, :, 0:P], wk3[:, :, 0:P])
            nc.sync.dma_start(yb[:, :, 0:P], y3[:, :, 0:P])
            # second wave, in order of first use (wv early: the VT chain
            # gates the whole AV pipeline)
            nc.sync.dma_start(yb[:, :, P:512], y3[:, :, P:512])
            nc.sync.dma_start(yb[:, :, 512:N], y3[:, :, 512:N])
            nc.sync.dma_start(wvb[:], wv3[:])
            nc.sync.dma_start(xb[:, :, 512:N], x3[:, :, 512:N])
            nc.sync.dma_start(wqb[:, :, P:C], wq3[:, :, P:C])
            nc.sync.dma_start(wkb[:, :, P:C], wk3[:, :, P:C])
            nc.sync.dma_start(wob[:], wo3[:])
            nc.sync.dma_start(bo_sb[:], bo.rearrange("(o p) -> p o", p=P))
            nc.sync.dma_start(bv_sb[:], bv[None, :])

            q_sb = [bigp.tile([P, N], BF16, name=f"q{t}", tag=f"q{t}") for t in range(CT)]
            k_sb = [bigp.tile([P, N], BF16, name=f"k{t}", tag=f"k{t}") for t in range(CT)]
            # VTa buffer: per n-tile, cols laid out [h][65] with col h*65+64 == 1.0
            vt_sb = [bigp.tile([P, NH * (DH + 1)], BF16, name=f"vt{t}", tag=f"vt{t}")
                     for t in range(QT)]
            for t in range(QT):
                nc.gpsimd.memset(vt_sb[t][:], 1.0)

            # PE warm-up: dummy matmuls while the input DMAs stream, so the
            # HAM clock gate is at 8/8 before the first projection and exp#1
            # isn't paying cold-clock prices (~3.4us of sustained PE activity
            # flips the gate)
            wup = psVT.tile([P, 512], F32, name="wup", tag="psvt")
            for i in range(18):
                nc.tensor.matmul(
                    wup[:], vt_sb[6][:, 0:P], vt_sb[7][:, 0:512],
                    start=True, stop=True,
                )

            # ---- projection helpers ----
            def qk_proj_cols(ct, dst, wb, srcb, nkt, bias_sb, c0, c1):
                """dst[:, c0:c1] = (W @ src)[ct*P:(ct+1)*P, c0:c1] + bias."""
                ps = psQK.tile([P, 512], F32, name="ps", tag="psqk")
                for kt in range(nkt):
                    nc.tensor.matmul(
                        ps[:, 0:c1 - c0],
                        wb[:, kt, ct * P:(ct + 1) * P],
                        srcb[:, kt, c0:c1],
                        start=(kt == 0),
                        stop=(kt == nkt - 1),
                    )
                nc.vector.tensor_scalar_add(
                    dst[:, c0:c1], ps[:, 0:c1 - c0], bias_sb[:, ct:ct + 1]
                )

            # ---- VT projection: VT[n, c] = sum_k y[k, n] * wvT[k, c]  (+ bias row)
            # medium priority (above AV/out-proj, below the exp chain): every
            # AV cell reads all eight vt tiles, so VT completing late gates
            # the whole AV pipeline. Groups alternate two pools so the
            # slot-release chain (MMs -> DVE copy -> next group) overlaps.
            def vt_proj(nt):
                with tc.high_priority(offset=HIPRI // 2):
                    pool, tag = [(psVT, "psvt"), (psQK, "psqk")][nt % 2]
                    ps = pool.tile([P, 512], F32, name="ps", tag=tag)
                    for kt in range(CCT):
                        nc.tensor.matmul(
                            ps[:],
                            yb[:, kt, nt * P:(nt + 1) * P],
                            wvb[:, kt, :],
                            start=(kt == 0),
                            stop=False,
                        )
                    nc.tensor.matmul(ps[:], ones_r[:], bv_sb[:], start=False, stop=True)
                    # scatter into [h][0:64] slots (col h*65+64 stays 1.0)
                    nc.vector.tensor_copy(
                        out=vt_sb[nt].rearrange("p (h e) -> p h e", e=DH + 1)[:, :, 0:DH],
                        in_=ps.rearrange("p (h d) -> p h d", d=DH),
                    )

            # pair-0 prologue: K in three column chunks chasing the y DMAs,
            # so exp#1 only waits on y strip0
            with tc.high_priority(offset=HIPRI):
                qk_proj_cols(0, q_sb[0], wqb, xb, CT, bq_sb, 0, 512)
                qk_proj_cols(0, k_sb[0], wkb, yb, CCT, bk_sb, 0, P)
                qk_proj_cols(0, k_sb[0], wkb, yb, CCT, bk_sb, P, 512)
                qk_proj_cols(0, q_sb[0], wqb, xb, CT, bq_sb, 512, N)
                qk_proj_cols(0, k_sb[0], wkb, yb, CCT, bk_sb, 512, N)

            # ---- attention ----
            qk_sb = [bigp.tile([P, N], BF16, name=f"qk{t}", tag=f"qk{t}")
                     for t in range(CT)]

            def ecol(hh, qt):
                return (qt // 4) * 1024 + hh * 512 + (qt % 4) * P

            def emit_av_bank(ph, hh, ets, bank, quirk_spread):
                """AV for one (head, 4-qt bank). Cell accumulation groups within
                one PSUM bank must be sequential (start=True clears has_written
                for the whole bank), so cells run kt-inner back-to-back."""
                h = 2 * ph + hh
                oa = oap.tile([P, 256], BF16, name="oa", tag="oa")
                av = psAV.tile([P, 512], F32, name="av", tag="av")
                for qq in range(4):
                    qt = bank * 4 + qq
                    for kt in range(QT):
                        nc.tensor.matmul(
                            av[:, qq * P:qq * P + DH + 1],
                            ets[kt][:, ecol(hh, qt):ecol(hh, qt) + P],
                            vt_sb[kt][:, h * (DH + 1):(h + 1) * (DH + 1)],
                            start=(kt == 0),
                            stop=(kt == QT - 1),
                        )
                # batched normalize: one reciprocal for the bank's 4
                # denominators, one broadcast multiply for all 4 cells
                rcp = rcpp.tile([P, 4], F32, name="rcp", tag="rcp")
                nc.vector.reciprocal(
                    rcp[:], av.rearrange("p (q c) -> p q c", c=P)[:, :, DH]
                )
                for qq in range(4):
                    nc.vector.tensor_scalar_mul(
                        oa[:, qq * DH:(qq + 1) * DH],
                        av[:, qq * P:qq * P + DH], rcp[:, qq:qq + 1],
                    )
                # quirk shuffle:
                # qk[ph][hh*64 + qt*8 + p//16, 64*(p%16)+d] = O_h[qt*128+p, d]
                for qq in range(4):
                    qt = bank * 4 + qq
                    eng = nc.scalar if (quirk_spread and qq % 2 == 1) else nc.sync
                    eng.dma_start(
                        qk_sb[ph][hh * 64 + qt * 8: hh * 64 + qt * 8 + 8, :],
                        oa[:, qq * DH:(qq + 1) * DH],
                    )

            for ph in range(NH // 2):
                ets = [etp.tile([P, 2048], name="et", tag="et", dtype=BF16)
                       for _ in range(QT)]
                last_pair = ph == NH // 2 - 1
                for half in range(2):
                    for kt in range(QT):
                        with tc.high_priority(offset=HIPRI):
                            sps = psS.tile([P, 1024], F32, name="sps", tag="pss")
                            for hh in range(2):
                                bp = hh * DH
                                nc.tensor.matmul(
                                    sps[:, hh * 512:(hh + 1) * 512],
                                    k_sb[ph][bp:bp + DH, kt * P:(kt + 1) * P],
                                    q_sb[ph][bp:bp + DH, half * 512:(half + 1) * 512],
                                    start=True,
                                    stop=True,
                                )
                            nc.scalar.activation(
                                ets[kt][:, half * 1024:(half + 1) * 1024], sps[:],
                                EXP, scale=0.125,
                            )
                        if ph == 0 and half == 0:
                            vt_proj(kt)  # before first reader (AV below)
                        # next-pair projections gate the next pair's exp chain
                        if ph + 1 < NH // 2:
                            np1 = ph + 1
                            u = half * 8 + kt
                            if u == 1:
                                with tc.high_priority(offset=HIPRI):
                                    qk_proj_cols(np1, q_sb[np1], wqb, xb, CT, bq_sb, 0, 512)
                            elif u == 3:
                                with tc.high_priority(offset=HIPRI):
                                    qk_proj_cols(np1, k_sb[np1], wkb, yb, CCT, bk_sb, 0, 512)
                            elif u == 9:
                                with tc.high_priority(offset=HIPRI):
                                    qk_proj_cols(np1, q_sb[np1], wqb, xb, CT, bq_sb, 512, N)
                            elif u == 11:
                                with tc.high_priority(offset=HIPRI):
                                    qk_proj_cols(np1, k_sb[np1], wkb, yb, CCT, bk_sb, 512, N)
                    # bank `half` only needs the ETs of this half: emit right
                    # after the half's last unit so it overlaps the other half
                    emit_av_bank(ph, 0, ets, half, quirk_spread=last_pair and half == 1)
                    emit_av_bank(ph, 1, ets, half, quirk_spread=last_pair and half == 1)

            # ---- output projection ----
            # ct0/ct1 groups take psQK/psVT (free during pairs 1-3, so their
            # kt=0..2 members pre-run); ct2/ct3 take psS (free after last exp)
            out3 = out.rearrange("(t p) n -> p t n", p=P)
            for ct in range(CT):
                for half in range(2):
                    pool, tag = [(psQK, "psqk"), (psVT, "psvt"),
                                 (psS, "pss"), (psS, "pss")][ct]
                    ps = pool.tile([P, 512], F32, name="ps", tag=tag)
                    for kt in range(CT):
                        nc.tensor.matmul(
                            ps[:, 0:512],
                            wob[:, kt, ct * P:(ct + 1) * P],
                            qk_sb[kt][:, half * 512:(half + 1) * 512],
                            start=(kt == 0),
                            stop=(kt == CT - 1),
                        )
                    ev = evp.tile([P, 512], BF16, name="ev", tag="ev")
                    nc.vector.tensor_scalar_add(ev[:], ps[:, 0:512], bo_sb[:, ct:ct + 1])
                    eng = nc.scalar if half == 1 else nc.sync
                    eng.dma_start(out3[:, ct, half * 512:(half + 1) * 512], ev[:])

    nc.compile()
    return nc


def kernel(**inputs) -> np.ndarray:
    bf = ml_dtypes.bfloat16
    x = np.ascontiguousarray(np.asarray(inputs["x"], dtype=np.float32).astype(bf))
    y = np.ascontiguousarray(np.asarray(inputs["y"], dtype=np.float32).astype(bf))
    wqT = np.ascontiguousarray(np.asarray(inputs["w_q"], dtype=np.float32).T.astype(bf))
    wkT = np.ascontiguousarray(np.asarray(inputs["w_k"], dtype=np.float32).T.astype(bf))
    wvT = np.ascontiguousarray(np.asarray(inputs["w_v"], dtype=np.float32).T.astype(bf))
    woT = np.ascontiguousarray(np.asarray(inputs["w_o"], dtype=np.float32).T.astype(bf))
    bq = np.ascontiguousarray(np.asarray(inputs["b_q"], dtype=np.float32))
    bk = np.ascontiguousarray(np.asarray(inputs["b_k"], dtype=np.float32))
    bv = np.ascontiguousarray(np.asarray(inputs["b_v"], dtype=np.float32).astype(bf))
    bo = np.ascontiguousarray(np.asarray(inputs["b_o"], dtype=np.float32))

    if "nc" not in _CACHE:
        _CACHE["nc"] = _build_nc()
    nc = _CACHE["nc"]

    in_maps = []
    for b in range(B):
        in_maps.append({
            "x": np.ascontiguousarray(x[b].reshape(C, N)),
            "y": np.ascontiguousarray(y[b].reshape(CC, N)),
            "wqT": wqT, "wkT": wkT, "wvT": wvT, "woT": woT,
            "bq": bq, "bk": bk, "bv": bv, "bo": bo,
        })
    res = run_bass_kernel_spmd(nc, in_maps, core_ids=list(range(B)))
    return np.stack([
        np.asarray(res.results[b]["out"]).astype(np.float32).reshape(C, HW, HW)
        for b in range(B)
    ])


# revision 17
# speedup vs baseline: 1.1466x; 1.0058x over previous
"""CrossAttention2D Trainium2 kernel (v4).

Sharding: data-parallel over batch. B=8 -> one batch element per NeuronCore,
no collectives. Weights replicated; host pre-transposes and casts to bf16.

Per-core math (C=512, Ccross=768, N=1024, 8 heads x 64):
  Q = Wq @ x_b          [C, N]   bf16
  K = Wk @ y_b          [C, N]   bf16
  VTa = [(Wv @ y_b).T | 1]       [N, 8*(64+1)] bf16 (ones col per head)
  per head pair ph (heads at PE rows 0/64, row-tiled scores):
    S[k, q] = K_h^T Q_h          psum [128, 1024] per (half, kt), ping-pong
    ET = exp(S/8)                ACT -> SBUF bf16, resident for whole pair
    O_aug[q, 0:65] = ET_tile^T @ VTa_h   (ET stationary, 65-col streams,
                                          kt-inner per cell; col 64 = denom)
    O = O_aug[:, :64] / O_aug[:, 64]     (DVE reciprocal + scale) -> bf16
  quirk: out_flat[h*64 + q//16, 64*(q%16) + d] = O_h[q, d]  (DMA shuffle)
  out = Wo @ quirk + bo          [C, N] -> bf16 out, host casts to f32

v4 scheduling (from v2/v3 ntff traces):
  - separate PSUM pools for next-pair Q/K proj (psQK) vs VT proj (psVT):
    in v2/v3 they shared one 2-slot pool, so the exp-critical Q/K proj
    serialized behind low-priority VT work -> 5-14us ACT stall per pair
  - exp-critical chain (Q/K proj, scores, exp) under tc.high_priority
  - consolidated 3D input DMAs (one instr per tensor slice group, ~600ns
    issue each) ordered so exp#1 only waits on ~1MB: wq-ct0, x-h0,
    wk-ct0, y-strip0; K proj for pair 0 follows the slices
  - AV emitted per (pair, bank): bank0 only needs half0 ETs
  - last pair's quirk + output DMAs alternate sync/scalar (ACT idle)
  - out-proj groups ct0/ct1 use psQK/psVT (free during pairs 1-3 -> they
    pre-run), ct2/ct3 use psS (free after the last exp)
"""

import numpy as np
import ml_dtypes

import concourse.bass as bass
import concourse.mybir as mybir
import concourse.tile as tile
from concourse import bacc
from concourse.bass_utils import run_bass_kernel_spmd

P = 128
C = 512          # d_embed
CC = 768         # d_cross
N = 1024         # H*W = 32*32
NH = 8
DH = 64
CT = C // P      # 4
CCT = CC // P    # 6
QT = N // P      # 8
HW = 32
B = 8
F32 = mybir.dt.float32
BF16 = mybir.dt.bfloat16
HIPRI = 1_000_000

_CACHE = {}


def _build_nc():
    nc = bacc.Bacc("TRN2", target_bir_lowering=False, debug=False, num_devices=B)

    x = nc.dram_tensor("x", [C, N], BF16, kind="ExternalInput")
    y = nc.dram_tensor("y", [CC, N], BF16, kind="ExternalInput")
    wqT = nc.dram_tensor("wqT", [C, C], BF16, kind="ExternalInput")
    wkT = nc.dram_tensor("wkT", [CC, C], BF16, kind="ExternalInput")
    wvT = nc.dram_tensor("wvT", [CC, C], BF16, kind="ExternalInput")
    woT = nc.dram_tensor("woT", [C, C], BF16, kind="ExternalInput")
    bq = nc.dram_tensor("bq", [C], F32, kind="ExternalInput")
    bk = nc.dram_tensor("bk", [C], F32, kind="ExternalInput")
    bv = nc.dram_tensor("bv", [C], BF16, kind="ExternalInput")
    bo = nc.dram_tensor("bo", [C], F32, kind="ExternalInput")
    out = nc.dram_tensor("out", [C, N], BF16, kind="ExternalOutput")

    EXP = mybir.ActivationFunctionType.Exp

    with tile.TileContext(nc) as tc:
        with (
            tc.tile_pool(name="const", bufs=1) as constp,
            tc.tile_pool(name="big", bufs=1) as bigp,
            tc.tile_pool(name="et", bufs=24) as etp,
            tc.tile_pool(name="oa", bufs=4) as oap,
            tc.tile_pool(name="ev", bufs=3) as evp,
            tc.tile_pool(name="rcp", bufs=4) as rcpp,
            tc.tile_pool(name="psS", bufs=2, space="PSUM") as psS,
            tc.tile_pool(name="psAV", bufs=2, space="PSUM") as psAV,
            tc.tile_pool(name="psQK", bufs=1, space="PSUM") as psQK,
            tc.tile_pool(name="psVT", bufs=1, space="PSUM") as psVT,
        ):
            # ---- constants ----
            ones_r = constp.tile([1, P], BF16, name="ones_r", tag="ones_r")
            nc.vector.memset(ones_r[:], 1.0)
            # preload the exp table set early so the ~2.7us ACT_TABLE_LOAD
            # overlaps the input DMA phase instead of the first real exp
            dmy = constp.tile([P, 1], F32, name="dmy", tag="dmy")
            nc.vector.memset(dmy[:], 0.0)
            dmy2 = constp.tile([P, 1], F32, name="dmy2", tag="dmy2")
            nc.scalar.activation(dmy2[:], dmy[:], EXP)

            bq_sb = constp.tile([P, CT], F32, name="bq", tag="bq")
            bk_sb = constp.tile([P, CT], F32, name="bk", tag="bk")
            bo_sb = constp.tile([P, CT], F32, name="bo", tag="bo")
            bv_sb = constp.tile([1, C], BF16, name="bv", tag="bv")

            # ---- consolidated input tiles (3D views) ----
            x3 = x.rearrange("(t p) n -> p t n", p=P)
            y3 = y.rearrange("(t p) n -> p t n", p=P)
            wq3 = wqT.rearrange("(t p) m -> p t m", p=P)
            wk3 = wkT.rearrange("(t p) m -> p t m", p=P)
            wv3 = wvT.rearrange("(t p) m -> p t m", p=P)
            wo3 = woT.rearrange("(t p) m -> p t m", p=P)

            xb = bigp.tile([P, CT, N], BF16, name="xb", tag="xb")
            yb = bigp.tile([P, CCT, N], BF16, name="yb", tag="yb")
            wqb = bigp.tile([P, CT, C], BF16, name="wqb", tag="wqb")
            wkb = bigp.tile([P, CCT, C], BF16, name="wkb", tag="wkb")
            wvb = bigp.tile([P, CCT, C], BF16, name="wvb", tag="wvb")
            wob = bigp.tile([P, CT, C], BF16, name="wob", tag="wob")

            # all input DMAs on ONE queue (sync) in strict priority order:
            # the DGE issue order is also the HBM bandwidth allocation order,
            # so eager second-wave DMAs must not race the exp#1-critical wave
            # (v4 lost ~5us to x-h0 sharing bandwidth with later loads)
            nc.scalar.dma_start(bq_sb[:], bq.rearrange("(o p) -> p o", p=P))
            nc.scalar.dma_start(bk_sb[:], bk.rearrange("(o p) -> p o", p=P))
            nc.sync.dma_start(wqb[:, :, 0:P], wq3[:, :, 0:P])
            nc.sync.dma_start(xb[:, :, 0:512], x3[:, :, 0:512])
            nc.sync.dma_start(wkb[:, :, 0:P], wk3[:, :, 0:P])
            nc.sync.dma_start(yb[:, :, 0:P], y3[:, :, 0:P])
            # second wave, in order of first use (wv early: the VT chain
            # gates the whole AV pipeline)
            nc.sync.dma_start(yb[:, :, P:512], y3[:, :, P:512])
            nc.sync.dma_start(yb[:, :, 512:N], y3[:, :, 512:N])
            nc.sync.dma_start(wvb[:], wv3[:])
            nc.sync.dma_start(xb[:, :, 512:N], x3[:, :, 512:N])
            nc.sync.dma_start(wqb[:, :, P:C], wq3[:, :, P:C])
            nc.sync.dma_start(wkb[:, :, P:C], wk3[:, :, P:C])
            nc.sync.dma_start(wob[:], wo3[:])
            nc.sync.dma_start(bo_sb[:], bo.rearrange("(o p) -> p o", p=P))
            nc.sync.dma_start(bv_sb[:], bv[None, :])

            q_sb = [bigp.tile([P, N], BF16, name=f"q{t}", tag=f"q{t}") for t in range(CT)]
            k_sb = [bigp.tile([P, N], BF16, name=f"k{t}", tag=f"k{t}") for t in range(CT)]
            # VTa buffer: per n-tile, cols laid out [h][65] with col h*65+64 == 1.0
            vt_sb = [bigp.tile([P, NH * (DH + 1)], BF16, name=f"vt{t}", tag=f"vt{t}")
                     for t in range(QT)]
            for t in range(QT):
                nc.gpsimd.memset(vt_sb[t][:], 1.0)

            # PE warm-up: dummy matmuls while the input DMAs stream, so the
            # HAM clock gate is at 8/8 before the first projection and exp#1
            # isn't paying cold-clock prices (~3.4us of sustained PE activity
            # flips the gate)
            wup = psVT.tile([P, 512], F32, name="wup", tag="psvt")
            for i in range(18):
                nc.tensor.matmul(
                    wup[:], vt_sb[6][:, 0:P], vt_sb[7][:, 0:512],
                    start=True, stop=True,
                )

            # ---- projection helpers ----
            def qk_proj_cols(ct, dst, wb, srcb, nkt, bias_sb, c0, c1):
                """dst[:, c0:c1] = (W @ src)[ct*P:(ct+1)*P, c0:c1] + bias."""
                ps = psQK.tile([P, 512], F32, name="ps", tag="psqk")
                for kt in range(nkt):
                    nc.tensor.matmul(
                        ps[:, 0:c1 - c0],
                        wb[:, kt, ct * P:(ct + 1) * P],
                        srcb[:, kt, c0:c1],
                        start=(kt == 0),
                        stop=(kt == nkt - 1),
                    )
                nc.vector.tensor_scalar_add(
                    dst[:, c0:c1], ps[:, 0:c1 - c0], bias_sb[:, ct:ct + 1]
                )

            # ---- VT projection: VT[n, c] = sum_k y[k, n] * wvT[k, c]  (+ bias row)
            # medium priority (above AV/out-proj, below the exp chain): every
            # AV cell reads all eight vt tiles, so VT completing late gates
            # the whole AV pipeline. Groups alternate two pools so the
            # slot-release chain (MMs -> DVE copy -> next group) overlaps.
            def vt_proj(nt):
                with tc.high_priority(offset=HIPRI // 2):
                    # psAV pool: idle until the first AV group (~35us), so VT
                    # double-buffers there without touching the exp-critical
                    # psQK pool
                    ps = psAV.tile([P, 512], F32, name="ps", tag="av")
                    for kt in range(CCT):
                        nc.tensor.matmul(
                            ps[:],
                            yb[:, kt, nt * P:(nt + 1) * P],
                            wvb[:, kt, :],
                            start=(kt == 0),
                            stop=False,
                        )
                    nc.tensor.matmul(ps[:], ones_r[:], bv_sb[:], start=False, stop=True)
                    # scatter into [h][0:64] slots (col h*65+64 stays 1.0)
                    nc.vector.tensor_copy(
                        out=vt_sb[nt].rearrange("p (h e) -> p h e", e=DH + 1)[:, :, 0:DH],
                        in_=ps.rearrange("p (h d) -> p h d", d=DH),
                    )

            # pair-0 prologue: K in three column chunks chasing the y DMAs,
            # so exp#1 only waits on y strip0
            with tc.high_priority(offset=HIPRI):
                qk_proj_cols(0, q_sb[0], wqb, xb, CT, bq_sb, 0, 512)
                qk_proj_cols(0, k_sb[0], wkb, yb, CCT, bk_sb, 0, P)
                qk_proj_cols(0, k_sb[0], wkb, yb, CCT, bk_sb, P, 512)
                qk_proj_cols(0, q_sb[0], wqb, xb, CT, bq_sb, 512, N)
                qk_proj_cols(0, k_sb[0], wkb, yb, CCT, bk_sb, 512, N)

            # ---- attention ----
            qk_sb = [bigp.tile([P, N], BF16, name=f"qk{t}", tag=f"qk{t}")
                     for t in range(CT)]

            def ecol(hh, qt):
                return (qt // 4) * 1024 + hh * 512 + (qt % 4) * P

            def emit_av_bank(ph, hh, ets, bank, quirk_spread):
                """AV for one (head, 4-qt bank). Cell accumulation groups within
                one PSUM bank must be sequential (start=True clears has_written
                for the whole bank), so cells run kt-inner back-to-back."""
                h = 2 * ph + hh
                oa = oap.tile([P, 256], BF16, name="oa", tag="oa")
                av = psAV.tile([P, 512], F32, name="av", tag="av")
                for qq in range(4):
                    qt = bank * 4 + qq
                    for kt in range(QT):
                        nc.tensor.matmul(
                            av[:, qq * P:qq * P + DH + 1],
                            ets[kt][:, ecol(hh, qt):ecol(hh, qt) + P],
                            vt_sb[kt][:, h * (DH + 1):(h + 1) * (DH + 1)],
                            start=(kt == 0),
                            stop=(kt == QT - 1),
                        )
                # batched normalize: one reciprocal for the bank's 4
                # denominators, one broadcast multiply for all 4 cells
                rcp = rcpp.tile([P, 4], F32, name="rcp", tag="rcp")
                nc.vector.reciprocal(
                    rcp[:], av.rearrange("p (q c) -> p q c", c=P)[:, :, DH]
                )
                for qq in range(4):
                    nc.vector.tensor_scalar_mul(
                        oa[:, qq * DH:(qq + 1) * DH],
                        av[:, qq * P:qq * P + DH], rcp[:, qq:qq + 1],
                    )
                # quirk shuffle:
                # qk[ph][hh*64 + qt*8 + p//16, 64*(p%16)+d] = O_h[qt*128+p, d]
                for qq in range(4):
                    qt = bank * 4 + qq
                    eng = nc.scalar if (quirk_spread and qq % 2 == 1) else nc.sync
                    eng.dma_start(
                        qk_sb[ph][hh * 64 + qt * 8: hh * 64 + qt * 8 + 8, :],
                        oa[:, qq * DH:(qq + 1) * DH],
                    )

            for ph in range(NH // 2):
                ets = [etp.tile([P, 2048], name="et", tag="et", dtype=BF16)
                       for _ in range(QT)]
                last_pair = ph == NH // 2 - 1
                for half in range(2):
                    for kt in range(QT):
                        with tc.high_priority(offset=HIPRI):
                            sps = psS.tile([P, 1024], F32, name="sps", tag="pss")
                            for hh in range(2):
                                bp = hh * DH
                                nc.tensor.matmul(
                                    sps[:, hh * 512:(hh + 1) * 512],
                                    k_sb[ph][bp:bp + DH, kt * P:(kt + 1) * P],
                                    q_sb[ph][bp:bp + DH, half * 512:(half + 1) * 512],
                                    start=True,
                                    stop=True,
                                )
                            nc.scalar.activation(
                                ets[kt][:, half * 1024:(half + 1) * 1024], sps[:],
                                EXP, scale=0.125,
                            )
                        if ph == 0 and half == 0:
                            vt_proj(kt)  # before first reader (AV below)
                        # next-pair projections gate the next pair's exp chain
                        if ph + 1 < NH // 2:
                            np1 = ph + 1
                            u = half * 8 + kt
                            if u == 1:
                                with tc.high_priority(offset=HIPRI):
                                    qk_proj_cols(np1, q_sb[np1], wqb, xb, CT, bq_sb, 0, 512)
                            elif u == 3:
                                with tc.high_priority(offset=HIPRI):
                                    qk_proj_cols(np1, k_sb[np1], wkb, yb, CCT, bk_sb, 0, 512)
                            elif u == 9:
                                with tc.high_priority(offset=HIPRI):
                                    qk_proj_cols(np1, q_sb[np1], wqb, xb, CT, bq_sb, 512, N)
                            elif u == 11:
                                with tc.high_priority(offset=HIPRI):
                                    qk_proj_cols(np1, k_sb[np1], wkb, yb, CCT, bk_sb, 512, N)
                    # bank `half` only needs the ETs of this half: emit right
                    # after the half's last unit so it overlaps the other half
                    emit_av_bank(ph, 0, ets, half, quirk_spread=last_pair and half == 1)
                    emit_av_bank(ph, 1, ets, half, quirk_spread=last_pair and half == 1)

            # ---- output projection ----
            # ct0/ct1 groups take psQK/psVT (free during pairs 1-3, so their
            # kt=0..2 members pre-run); ct2/ct3 take psS (free after last exp)
            out3 = out.rearrange("(t p) n -> p t n", p=P)
            for ct in range(CT):
                for half in range(2):
                    pool, tag = [(psQK, "psqk"), (psVT, "psvt"),
                                 (psS, "pss"), (psS, "pss")][ct]
                    ps = pool.tile([P, 512], F32, name="ps", tag=tag)
                    for kt in range(CT):
                        nc.tensor.matmul(
                            ps[:, 0:512],
                            wob[:, kt, ct * P:(ct + 1) * P],
                            qk_sb[kt][:, half * 512:(half + 1) * 512],
                            start=(kt == 0),
                            stop=(kt == CT - 1),
                        )
                    ev = evp.tile([P, 512], BF16, name="ev", tag="ev")
                    nc.vector.tensor_scalar_add(ev[:], ps[:, 0:512], bo_sb[:, ct:ct + 1])
                    eng = nc.scalar if half == 1 else nc.sync
                    eng.dma_start(out3[:, ct, half * 512:(half + 1) * 512], ev[:])

    nc.compile()
    return nc


def kernel(**inputs) -> np.ndarray:
    bf = ml_dtypes.bfloat16
    x = np.ascontiguousarray(np.asarray(inputs["x"], dtype=np.float32).astype(bf))
    y = np.ascontiguousarray(np.asarray(inputs["y"], dtype=np.float32).astype(bf))
    wqT = np.ascontiguousarray(np.asarray(inputs["w_q"], dtype=np.float32).T.astype(bf))
    wkT = np.ascontiguousarray(np.asarray(inputs["w_k"], dtype=np.float32).T.astype(bf))
    wvT = np.ascontiguousarray(np.asarray(inputs["w_v"], dtype=np.float32).T.astype(bf))
    woT = np.ascontiguousarray(np.asarray(inputs["w_o"], dtype=np.float32).T.astype(bf))
    bq = np.ascontiguousarray(np.asarray(inputs["b_q"], dtype=np.float32))
    bk = np.ascontiguousarray(np.asarray(inputs["b_k"], dtype=np.float32))
    bv = np.ascontiguousarray(np.asarray(inputs["b_v"], dtype=np.float32).astype(bf))
    bo = np.ascontiguousarray(np.asarray(inputs["b_o"], dtype=np.float32))

    if "nc" not in _CACHE:
        _CACHE["nc"] = _build_nc()
    nc = _CACHE["nc"]

    in_maps = []
    for b in range(B):
        in_maps.append({
            "x": np.ascontiguousarray(x[b].reshape(C, N)),
            "y": np.ascontiguousarray(y[b].reshape(CC, N)),
            "wqT": wqT, "wkT": wkT, "wvT": wvT, "woT": woT,
            "bq": bq, "bk": bk, "bv": bv, "bo": bo,
        })
    res = run_bass_kernel_spmd(nc, in_maps, core_ids=list(range(B)))
    return np.stack([
        np.asarray(res.results[b]["out"]).astype(np.float32).reshape(C, HW, HW)
        for b in range(B)
    ])


# revision 23
# speedup vs baseline: 1.1553x; 1.0076x over previous
"""CrossAttention2D Trainium2 kernel (v4).

Sharding: data-parallel over batch. B=8 -> one batch element per NeuronCore,
no collectives. Weights replicated; host pre-transposes and casts to bf16.

Per-core math (C=512, Ccross=768, N=1024, 8 heads x 64):
  Q = Wq @ x_b          [C, N]   bf16
  K = Wk @ y_b          [C, N]   bf16
  VTa = [(Wv @ y_b).T | 1]       [N, 8*(64+1)] bf16 (ones col per head)
  per head pair ph (heads at PE rows 0/64, row-tiled scores):
    S[k, q] = K_h^T Q_h          psum [128, 1024] per (half, kt), ping-pong
    ET = exp(S/8)                ACT -> SBUF bf16, resident for whole pair
    O_aug[q, 0:65] = ET_tile^T @ VTa_h   (ET stationary, 65-col streams,
                                          kt-inner per cell; col 64 = denom)
    O = O_aug[:, :64] / O_aug[:, 64]     (DVE reciprocal + scale) -> bf16
  quirk: out_flat[h*64 + q//16, 64*(q%16) + d] = O_h[q, d]  (DMA shuffle)
  out = Wo @ quirk + bo          [C, N] -> bf16 out, host casts to f32

v4 scheduling (from v2/v3 ntff traces):
  - separate PSUM pools for next-pair Q/K proj (psQK) vs VT proj (psVT):
    in v2/v3 they shared one 2-slot pool, so the exp-critical Q/K proj
    serialized behind low-priority VT work -> 5-14us ACT stall per pair
  - exp-critical chain (Q/K proj, scores, exp) under tc.high_priority
  - consolidated 3D input DMAs (one instr per tensor slice group, ~600ns
    issue each) ordered so exp#1 only waits on ~1MB: wq-ct0, x-h0,
    wk-ct0, y-strip0; K proj for pair 0 follows the slices
  - AV emitted per (pair, bank): bank0 only needs half0 ETs
  - last pair's quirk + output DMAs alternate sync/scalar (ACT idle)
  - out-proj groups ct0/ct1 use psQK/psVT (free during pairs 1-3 -> they
    pre-run), ct2/ct3 use psS (free after the last exp)
"""

import numpy as np
import ml_dtypes

import concourse.bass as bass
import concourse.mybir as mybir
import concourse.tile as tile
from concourse import bacc
from concourse.bass_utils import run_bass_kernel_spmd

P = 128
C = 512          # d_embed
CC = 768         # d_cross
N = 1024         # H*W = 32*32
NH = 8
DH = 64
CT = C // P      # 4
CCT = CC // P    # 6
QT = N // P      # 8
HW = 32
B = 8
F32 = mybir.dt.float32
BF16 = mybir.dt.bfloat16
HIPRI = 1_000_000

_CACHE = {}


def _build_nc():
    nc = bacc.Bacc("TRN2", target_bir_lowering=False, debug=False, num_devices=B)

    x = nc.dram_tensor("x", [C, N], BF16, kind="ExternalInput")
    y = nc.dram_tensor("y", [CC, N], BF16, kind="ExternalInput")
    wqT = nc.dram_tensor("wqT", [C, C], BF16, kind="ExternalInput")
    wkT = nc.dram_tensor("wkT", [CC, C], BF16, kind="ExternalInput")
    wvT = nc.dram_tensor("wvT", [CC, C], BF16, kind="ExternalInput")
    woT = nc.dram_tensor("woT", [C, C], BF16, kind="ExternalInput")
    bq = nc.dram_tensor("bq", [C], F32, kind="ExternalInput")
    bk = nc.dram_tensor("bk", [C], F32, kind="ExternalInput")
    bv = nc.dram_tensor("bv", [C], BF16, kind="ExternalInput")
    bo = nc.dram_tensor("bo", [C], F32, kind="ExternalInput")
    out = nc.dram_tensor("out", [C, N], BF16, kind="ExternalOutput")

    EXP = mybir.ActivationFunctionType.Exp

    with tile.TileContext(nc) as tc:
        with (
            tc.tile_pool(name="const", bufs=1) as constp,
            tc.tile_pool(name="big", bufs=1) as bigp,
            tc.tile_pool(name="et", bufs=24) as etp,
            tc.tile_pool(name="oa", bufs=4) as oap,
            tc.tile_pool(name="ev", bufs=3) as evp,
            tc.tile_pool(name="rcp", bufs=4) as rcpp,
            tc.tile_pool(name="psS", bufs=2, space="PSUM") as psS,
            tc.tile_pool(name="psAV", bufs=3, space="PSUM") as psAV,
            tc.tile_pool(name="psQK", bufs=1, space="PSUM") as psQK,
        ):
            # ---- constants ----
            ones_r = constp.tile([1, P], BF16, name="ones_r", tag="ones_r")
            nc.vector.memset(ones_r[:], 1.0)
            # preload the exp table set early so the ~2.7us ACT_TABLE_LOAD
            # overlaps the input DMA phase instead of the first real exp
            dmy = constp.tile([P, 1], F32, name="dmy", tag="dmy")
            nc.vector.memset(dmy[:], 0.0)
            dmy2 = constp.tile([P, 1], F32, name="dmy2", tag="dmy2")
            nc.scalar.activation(dmy2[:], dmy[:], EXP)

            bq_sb = constp.tile([P, CT], F32, name="bq", tag="bq")
            bk_sb = constp.tile([P, CT], F32, name="bk", tag="bk")
            bo_sb = constp.tile([P, CT], F32, name="bo", tag="bo")
            bv_sb = constp.tile([1, C], BF16, name="bv", tag="bv")

            # ---- consolidated input tiles (3D views) ----
            x3 = x.rearrange("(t p) n -> p t n", p=P)
            y3 = y.rearrange("(t p) n -> p t n", p=P)
            wq3 = wqT.rearrange("(t p) m -> p t m", p=P)
            wk3 = wkT.rearrange("(t p) m -> p t m", p=P)
            wv3 = wvT.rearrange("(t p) m -> p t m", p=P)
            wo3 = woT.rearrange("(t p) m -> p t m", p=P)

            xb = bigp.tile([P, CT, N], BF16, name="xb", tag="xb")
            yb = bigp.tile([P, CCT, N], BF16, name="yb", tag="yb")
            wqb = bigp.tile([P, CT, C], BF16, name="wqb", tag="wqb")
            wkb = bigp.tile([P, CCT, C], BF16, name="wkb", tag="wkb")
            wvb = bigp.tile([P, CCT, C], BF16, name="wvb", tag="wvb")
            wob = bigp.tile([P, CT, C], BF16, name="wob", tag="wob")

            # all input DMAs on ONE queue (sync) in strict priority order:
            # the DGE issue order is also the HBM bandwidth allocation order,
            # so eager second-wave DMAs must not race the exp#1-critical wave
            # (v4 lost ~5us to x-h0 sharing bandwidth with later loads)
            nc.scalar.dma_start(bq_sb[:], bq.rearrange("(o p) -> p o", p=P))
            nc.scalar.dma_start(bk_sb[:], bk.rearrange("(o p) -> p o", p=P))
            nc.sync.dma_start(wqb[:, :, 0:P], wq3[:, :, 0:P])
            nc.sync.dma_start(xb[:, :, 0:512], x3[:, :, 0:512])
            nc.sync.dma_start(wkb[:, :, 0:P], wk3[:, :, 0:P])
            nc.sync.dma_start(yb[:, :, 0:P], y3[:, :, 0:P])
            # second wave, in order of first use (wv early: the VT chain
            # gates the whole AV pipeline; y half1 split so K cols 512-767
            # are projectable sooner)
            nc.sync.dma_start(yb[:, :, P:512], y3[:, :, P:512])
            nc.sync.dma_start(yb[:, :, 512:768], y3[:, :, 512:768])
            nc.sync.dma_start(yb[:, :, 768:N], y3[:, :, 768:N])
            nc.sync.dma_start(wvb[:], wv3[:])
            nc.sync.dma_start(xb[:, :, 512:N], x3[:, :, 512:N])
            nc.sync.dma_start(wqb[:, :, P:C], wq3[:, :, P:C])
            nc.sync.dma_start(wkb[:, :, P:C], wk3[:, :, P:C])
            nc.sync.dma_start(wob[:], wo3[:])
            nc.sync.dma_start(bo_sb[:], bo.rearrange("(o p) -> p o", p=P))
            nc.sync.dma_start(bv_sb[:], bv[None, :])

            q_sb = [bigp.tile([P, N], BF16, name=f"q{t}", tag=f"q{t}") for t in range(CT)]
            k_sb = [bigp.tile([P, N], BF16, name=f"k{t}", tag=f"k{t}") for t in range(CT)]
            # VTa buffer: per n-tile, cols laid out [h][65] with col h*65+64 == 1.0
            vt_sb = [bigp.tile([P, NH * (DH + 1)], BF16, name=f"vt{t}", tag=f"vt{t}")
                     for t in range(QT)]
            for t in range(QT):
                nc.gpsimd.memset(vt_sb[t][:], 1.0)

            # PE warm-up: dummy matmuls while the input DMAs stream, so the
            # HAM clock gate is at 8/8 before the first projection and exp#1
            # isn't paying cold-clock prices (~3.4us of sustained PE activity
            # flips the gate)
            wup = psAV.tile([P, 512], F32, name="wup", tag="av")
            for i in range(18):
                nc.tensor.matmul(
                    wup[:], vt_sb[6][:, 0:P], vt_sb[7][:, 0:512],
                    start=True, stop=True,
                )

            # ---- projection helpers ----
            def qk_proj_cols(ct, dst, wb, srcb, nkt, bias_sb, c0, c1):
                """dst[:, c0:c1] = (W @ src)[ct*P:(ct+1)*P, c0:c1] + bias."""
                ps = psQK.tile([P, 512], F32, name="ps", tag="psqk")
                for kt in range(nkt):
                    nc.tensor.matmul(
                        ps[:, 0:c1 - c0],
                        wb[:, kt, ct * P:(ct + 1) * P],
                        srcb[:, kt, c0:c1],
                        start=(kt == 0),
                        stop=(kt == nkt - 1),
                    )
                nc.vector.tensor_scalar_add(
                    dst[:, c0:c1], ps[:, 0:c1 - c0], bias_sb[:, ct:ct + 1]
                )

            # ---- VT projection: VT[n, c] = sum_k y[k, n] * wvT[k, c]  (+ bias row)
            # medium priority (above AV/out-proj, below the exp chain): every
            # AV cell reads all eight vt tiles, so VT completing late gates
            # the whole AV pipeline. Groups alternate two pools so the
            # slot-release chain (MMs -> DVE copy -> next group) overlaps.
            def vt_proj(nt):
                with tc.high_priority(offset=HIPRI // 2):
                    # psAV pool: idle until the first AV group (~35us), so VT
                    # double-buffers there without touching the exp-critical
                    # psQK pool
                    ps = psAV.tile([P, 512], F32, name="ps", tag="av")
                    for kt in range(CCT):
                        nc.tensor.matmul(
                            ps[:],
                            yb[:, kt, nt * P:(nt + 1) * P],
                            wvb[:, kt, :],
                            start=(kt == 0),
                            stop=False,
                        )
                    nc.tensor.matmul(ps[:], ones_r[:], bv_sb[:], start=False, stop=True)
                    # scatter into [h][0:64] slots (col h*65+64 stays 1.0)
                    nc.vector.tensor_copy(
                        out=vt_sb[nt].rearrange("p (h e) -> p h e", e=DH + 1)[:, :, 0:DH],
                        in_=ps.rearrange("p (h d) -> p h d", d=DH),
                    )

            # pair-0 prologue: K in three column chunks chasing the y DMAs,
            # so exp#1 only waits on y strip0
            with tc.high_priority(offset=HIPRI):
                qk_proj_cols(0, q_sb[0], wqb, xb, CT, bq_sb, 0, 512)
                qk_proj_cols(0, k_sb[0], wkb, yb, CCT, bk_sb, 0, P)
                qk_proj_cols(0, k_sb[0], wkb, yb, CCT, bk_sb, P, 512)
                qk_proj_cols(0, k_sb[0], wkb, yb, CCT, bk_sb, 512, 768)
                qk_proj_cols(0, q_sb[0], wqb, xb, CT, bq_sb, 512, N)
                qk_proj_cols(0, k_sb[0], wkb, yb, CCT, bk_sb, 768, N)

            # ---- attention ----
            qk_sb = [bigp.tile([P, N], BF16, name=f"qk{t}", tag=f"qk{t}")
                     for t in range(CT)]

            def ecol(hh, qt):
                return (qt // 4) * 1024 + hh * 512 + (qt % 4) * P

            def emit_av_bank(ph, hh, ets, bank, quirk_spread):
                """AV for one (head, 4-qt bank). Cell accumulation groups within
                one PSUM bank must be sequential (start=True clears has_written
                for the whole bank), so cells run kt-inner back-to-back."""
                h = 2 * ph + hh
                oa = oap.tile([P, 256], BF16, name="oa", tag="oa")
                av = psAV.tile([P, 512], F32, name="av", tag="av")
                for qq in range(4):
                    qt = bank * 4 + qq
                    for kt in range(QT):
                        nc.tensor.matmul(
                            av[:, qq * P:qq * P + DH + 1],
                            ets[kt][:, ecol(hh, qt):ecol(hh, qt) + P],
                            vt_sb[kt][:, h * (DH + 1):(h + 1) * (DH + 1)],
                            start=(kt == 0),
                            stop=(kt == QT - 1),
                        )
                # batched normalize: one reciprocal for the bank's 4
                # denominators, then per-cell scale + shuffle. Medium
                # priority: the psAV slot only frees after these reads, so
                # they pace the whole AV chain.
                with tc.high_priority(offset=HIPRI // 2):
                    rcp = rcpp.tile([P, 4], F32, name="rcp", tag="rcp")
                    nc.vector.reciprocal(
                        rcp[:], av.rearrange("p (q c) -> p q c", c=P)[:, :, DH]
                    )
                    for qq in range(4):
                        nc.vector.tensor_scalar_mul(
                            oa[:, qq * DH:(qq + 1) * DH],
                            av[:, qq * P:qq * P + DH], rcp[:, qq:qq + 1],
                        )
                # quirk shuffle:
                # qk[ph][hh*64 + qt*8 + p//16, 64*(p%16)+d] = O_h[qt*128+p, d]
                for qq in range(4):
                    qt = bank * 4 + qq
                    eng = nc.scalar if (quirk_spread and qq % 2 == 1) else nc.sync
                    eng.dma_start(
                        qk_sb[ph][hh * 64 + qt * 8: hh * 64 + qt * 8 + 8, :],
                        oa[:, qq * DH:(qq + 1) * DH],
                    )

            for ph in range(NH // 2):
                ets = [etp.tile([P, 2048], name="et", tag="et", dtype=BF16)
                       for _ in range(QT)]
                last_pair = ph == NH // 2 - 1
                for half in range(2):
                    for kt in range(QT):
                        with tc.high_priority(offset=HIPRI):
                            sps = psS.tile([P, 1024], F32, name="sps", tag="pss")
                            for hh in range(2):
                                bp = hh * DH
                                nc.tensor.matmul(
                                    sps[:, hh * 512:(hh + 1) * 512],
                                    k_sb[ph][bp:bp + DH, kt * P:(kt + 1) * P],
                                    q_sb[ph][bp:bp + DH, half * 512:(half + 1) * 512],
                                    start=True,
                                    stop=True,
                                )
                            nc.scalar.activation(
                                ets[kt][:, half * 1024:(half + 1) * 1024], sps[:],
                                EXP, scale=0.125,
                            )
                        if ph == 0 and half == 0:
                            vt_proj(kt)  # before first reader (AV below)
                        # next-pair projections gate the next pair's exp chain
                        if ph + 1 < NH // 2:
                            np1 = ph + 1
                            u = half * 8 + kt
                            if u == 1:
                                with tc.high_priority(offset=HIPRI):
                                    qk_proj_cols(np1, q_sb[np1], wqb, xb, CT, bq_sb, 0, 512)
                            elif u == 3:
                                with tc.high_priority(offset=HIPRI):
                                    qk_proj_cols(np1, k_sb[np1], wkb, yb, CCT, bk_sb, 0, 512)
                            elif u == 9:
                                with tc.high_priority(offset=HIPRI):
                                    qk_proj_cols(np1, q_sb[np1], wqb, xb, CT, bq_sb, 512, N)
                            elif u == 11:
                                with tc.high_priority(offset=HIPRI):
                                    qk_proj_cols(np1, k_sb[np1], wkb, yb, CCT, bk_sb, 512, N)
                    # bank `half` only needs the ETs of this half: emit right
                    # after the half's last unit so it overlaps the other half
                    emit_av_bank(ph, 0, ets, half, quirk_spread=last_pair and half == 1)
                    emit_av_bank(ph, 1, ets, half, quirk_spread=last_pair and half == 1)

            # ---- output projection ----
            # ct0/ct1 groups take psQK/psVT (free during pairs 1-3, so their
            # kt=0..2 members pre-run); ct2/ct3 take psS (free after last exp)
            out3 = out.rearrange("(t p) n -> p t n", p=P)
            for ct in range(CT):
                for half in range(2):
                    pool, tag = [(psQK, "psqk"), (psQK, "psqk"),
                                 (psS, "pss"), (psS, "pss")][ct]
                    ps = pool.tile([P, 512], F32, name="ps", tag=tag)
                    for kt in range(CT):
                        nc.tensor.matmul(
                            ps[:, 0:512],
                            wob[:, kt, ct * P:(ct + 1) * P],
                            qk_sb[kt][:, half * 512:(half + 1) * 512],
                            start=(kt == 0),
                            stop=(kt == CT - 1),
                        )
                    ev = evp.tile([P, 512], BF16, name="ev", tag="ev")
                    nc.vector.tensor_scalar_add(ev[:], ps[:, 0:512], bo_sb[:, ct:ct + 1])
                    eng = nc.scalar if half == 1 else nc.sync
                    eng.dma_start(out3[:, ct, half * 512:(half + 1) * 512], ev[:])

    nc.compile()
    return nc


def kernel(**inputs) -> np.ndarray:
    bf = ml_dtypes.bfloat16
    x = np.ascontiguousarray(np.asarray(inputs["x"], dtype=np.float32).astype(bf))
    y = np.ascontiguousarray(np.asarray(inputs["y"], dtype=np.float32).astype(bf))
    wqT = np.ascontiguousarray(np.asarray(inputs["w_q"], dtype=np.float32).T.astype(bf))
    wkT = np.ascontiguousarray(np.asarray(inputs["w_k"], dtype=np.float32).T.astype(bf))
    wvT = np.ascontiguousarray(np.asarray(inputs["w_v"], dtype=np.float32).T.astype(bf))
    woT = np.ascontiguousarray(np.asarray(inputs["w_o"], dtype=np.float32).T.astype(bf))
    bq = np.ascontiguousarray(np.asarray(inputs["b_q"], dtype=np.float32))
    bk = np.ascontiguousarray(np.asarray(inputs["b_k"], dtype=np.float32))
    bv = np.ascontiguousarray(np.asarray(inputs["b_v"], dtype=np.float32).astype(bf))
    bo = np.ascontiguousarray(np.asarray(inputs["b_o"], dtype=np.float32))

    if "nc" not in _CACHE:
        _CACHE["nc"] = _build_nc()
    nc = _CACHE["nc"]

    in_maps = []
    for b in range(B):
        in_maps.append({
            "x": np.ascontiguousarray(x[b].reshape(C, N)),
            "y": np.ascontiguousarray(y[b].reshape(CC, N)),
            "wqT": wqT, "wkT": wkT, "wvT": wvT, "woT": woT,
            "bq": bq, "bk": bk, "bv": bv, "bo": bo,
        })
    res = run_bass_kernel_spmd(nc, in_maps, core_ids=list(range(B)))
    return np.stack([
        np.asarray(res.results[b]["out"]).astype(np.float32).reshape(C, HW, HW)
        for b in range(B)
    ])


# revision 24
# speedup vs baseline: 1.1739x; 1.0162x over previous
"""CrossAttention2D Trainium2 kernel (v4).

Sharding: data-parallel over batch. B=8 -> one batch element per NeuronCore,
no collectives. Weights replicated; host pre-transposes and casts to bf16.

Per-core math (C=512, Ccross=768, N=1024, 8 heads x 64):
  Q = Wq @ x_b          [C, N]   bf16
  K = Wk @ y_b          [C, N]   bf16
  VTa = [(Wv @ y_b).T | 1]       [N, 8*(64+1)] bf16 (ones col per head)
  per head pair ph (heads at PE rows 0/64, row-tiled scores):
    S[k, q] = K_h^T Q_h          psum [128, 1024] per (half, kt), ping-pong
    ET = exp(S/8)                ACT -> SBUF bf16, resident for whole pair
    O_aug[q, 0:65] = ET_tile^T @ VTa_h   (ET stationary, 65-col streams,
                                          kt-inner per cell; col 64 = denom)
    O = O_aug[:, :64] / O_aug[:, 64]     (DVE reciprocal + scale) -> bf16
  quirk: out_flat[h*64 + q//16, 64*(q%16) + d] = O_h[q, d]  (DMA shuffle)
  out = Wo @ quirk + bo          [C, N] -> bf16 out, host casts to f32

v4 scheduling (from v2/v3 ntff traces):
  - separate PSUM pools for next-pair Q/K proj (psQK) vs VT proj (psVT):
    in v2/v3 they shared one 2-slot pool, so the exp-critical Q/K proj
    serialized behind low-priority VT work -> 5-14us ACT stall per pair
  - exp-critical chain (Q/K proj, scores, exp) under tc.high_priority
  - consolidated 3D input DMAs (one instr per tensor slice group, ~600ns
    issue each) ordered so exp#1 only waits on ~1MB: wq-ct0, x-h0,
    wk-ct0, y-strip0; K proj for pair 0 follows the slices
  - AV emitted per (pair, bank): bank0 only needs half0 ETs
  - last pair's quirk + output DMAs alternate sync/scalar (ACT idle)
  - out-proj groups ct0/ct1 use psQK/psVT (free during pairs 1-3 -> they
    pre-run), ct2/ct3 use psS (free after the last exp)
"""

import numpy as np
import ml_dtypes

import concourse.bass as bass
import concourse.mybir as mybir
import concourse.tile as tile
from concourse import bacc
from concourse.bass_utils import run_bass_kernel_spmd

P = 128
C = 512          # d_embed
CC = 768         # d_cross
N = 1024         # H*W = 32*32
NH = 8
DH = 64
CT = C // P      # 4
CCT = CC // P    # 6
QT = N // P      # 8
HW = 32
B = 8
F32 = mybir.dt.float32
BF16 = mybir.dt.bfloat16
HIPRI = 1_000_000

_CACHE = {}


def _build_nc():
    nc = bacc.Bacc("TRN2", target_bir_lowering=False, debug=False, num_devices=B)

    x = nc.dram_tensor("x", [C, N], BF16, kind="ExternalInput")
    y = nc.dram_tensor("y", [CC, N], BF16, kind="ExternalInput")
    wqT = nc.dram_tensor("wqT", [C, C], BF16, kind="ExternalInput")
    wkT = nc.dram_tensor("wkT", [CC, C], BF16, kind="ExternalInput")
    wvT = nc.dram_tensor("wvT", [CC, C], BF16, kind="ExternalInput")
    woT = nc.dram_tensor("woT", [C, C], BF16, kind="ExternalInput")
    bq = nc.dram_tensor("bq", [C], F32, kind="ExternalInput")
    bk = nc.dram_tensor("bk", [C], F32, kind="ExternalInput")
    bv = nc.dram_tensor("bv", [C], BF16, kind="ExternalInput")
    bo = nc.dram_tensor("bo", [C], F32, kind="ExternalInput")
    out = nc.dram_tensor("out", [C, N], BF16, kind="ExternalOutput")

    EXP = mybir.ActivationFunctionType.Exp

    with tile.TileContext(nc) as tc:
        with (
            tc.tile_pool(name="const", bufs=1) as constp,
            tc.tile_pool(name="big", bufs=1) as bigp,
            tc.tile_pool(name="et", bufs=24) as etp,
            tc.tile_pool(name="oa", bufs=4) as oap,
            tc.tile_pool(name="ev", bufs=3) as evp,
            tc.tile_pool(name="rcp", bufs=4) as rcpp,
            tc.tile_pool(name="psS", bufs=2, space="PSUM") as psS,
            tc.tile_pool(name="psAV", bufs=3, space="PSUM") as psAV,
            tc.tile_pool(name="psQK", bufs=1, space="PSUM") as psQK,
        ):
            # ---- constants ----
            ones_r = constp.tile([1, P], BF16, name="ones_r", tag="ones_r")
            nc.vector.memset(ones_r[:], 1.0)
            # preload the exp table set early so the ~2.7us ACT_TABLE_LOAD
            # overlaps the input DMA phase instead of the first real exp
            dmy = constp.tile([P, 1], F32, name="dmy", tag="dmy")
            nc.vector.memset(dmy[:], 0.0)
            dmy2 = constp.tile([P, 1], F32, name="dmy2", tag="dmy2")
            nc.scalar.activation(dmy2[:], dmy[:], EXP)

            bq_sb = constp.tile([P, CT], F32, name="bq", tag="bq")
            bk_sb = constp.tile([P, CT], F32, name="bk", tag="bk")
            bo_sb = constp.tile([P, CT], F32, name="bo", tag="bo")
            bv_sb = constp.tile([1, C], BF16, name="bv", tag="bv")

            # ---- consolidated input tiles (3D views) ----
            x3 = x.rearrange("(t p) n -> p t n", p=P)
            y3 = y.rearrange("(t p) n -> p t n", p=P)
            wq3 = wqT.rearrange("(t p) m -> p t m", p=P)
            wk3 = wkT.rearrange("(t p) m -> p t m", p=P)
            wv3 = wvT.rearrange("(t p) m -> p t m", p=P)
            wo3 = woT.rearrange("(t p) m -> p t m", p=P)

            xb = bigp.tile([P, CT, N], BF16, name="xb", tag="xb")
            yb = bigp.tile([P, CCT, N], BF16, name="yb", tag="yb")
            wqb = bigp.tile([P, CT, C], BF16, name="wqb", tag="wqb")
            wkb = bigp.tile([P, CCT, C], BF16, name="wkb", tag="wkb")
            wvb = bigp.tile([P, CCT, C], BF16, name="wvb", tag="wvb")
            wob = bigp.tile([P, CT, C], BF16, name="wob", tag="wob")

            # all input DMAs on ONE queue (sync) in strict priority order:
            # the DGE issue order is also the HBM bandwidth allocation order,
            # so eager second-wave DMAs must not race the exp#1-critical wave
            # (v4 lost ~5us to x-h0 sharing bandwidth with later loads)
            nc.scalar.dma_start(bq_sb[:], bq.rearrange("(o p) -> p o", p=P))
            nc.scalar.dma_start(bk_sb[:], bk.rearrange("(o p) -> p o", p=P))
            nc.sync.dma_start(wqb[:, :, 0:P], wq3[:, :, 0:P])
            nc.sync.dma_start(xb[:, :, 0:512], x3[:, :, 0:512])
            nc.sync.dma_start(wkb[:, :, 0:P], wk3[:, :, 0:P])
            nc.sync.dma_start(yb[:, :, 0:P], y3[:, :, 0:P])
            # second wave, in order of first use (wv early: the VT chain
            # gates the whole AV pipeline; y half1 split so K cols 512-767
            # are projectable sooner)
            nc.sync.dma_start(yb[:, :, P:512], y3[:, :, P:512])
            nc.sync.dma_start(yb[:, :, 512:768], y3[:, :, 512:768])
            nc.sync.dma_start(yb[:, :, 768:N], y3[:, :, 768:N])
            nc.sync.dma_start(wvb[:], wv3[:])
            nc.sync.dma_start(xb[:, :, 512:N], x3[:, :, 512:N])
            nc.sync.dma_start(wqb[:, :, P:C], wq3[:, :, P:C])
            nc.sync.dma_start(wkb[:, :, P:C], wk3[:, :, P:C])
            nc.sync.dma_start(wob[:], wo3[:])
            nc.sync.dma_start(bo_sb[:], bo.rearrange("(o p) -> p o", p=P))
            nc.sync.dma_start(bv_sb[:], bv[None, :])

            q_sb = [bigp.tile([P, N], BF16, name=f"q{t}", tag=f"q{t}") for t in range(CT)]
            k_sb = [bigp.tile([P, N], BF16, name=f"k{t}", tag=f"k{t}") for t in range(CT)]
            # VTa buffer: per n-tile, cols laid out [h][65] with col h*65+64 == 1.0
            vt_sb = [bigp.tile([P, NH * (DH + 1)], BF16, name=f"vt{t}", tag=f"vt{t}")
                     for t in range(QT)]
            for t in range(QT):
                nc.gpsimd.memset(vt_sb[t][:], 1.0)

            # PE warm-up: dummy matmuls while the input DMAs stream, so the
            # HAM clock gate is at 8/8 before the first projection and exp#1
            # isn't paying cold-clock prices (~3.4us of sustained PE activity
            # flips the gate)
            wup = psAV.tile([P, 512], F32, name="wup", tag="av")
            for i in range(18):
                nc.tensor.matmul(
                    wup[:], vt_sb[6][:, 0:P], vt_sb[7][:, 0:512],
                    start=True, stop=True,
                )

            # ---- projection helpers ----
            def qk_proj_cols(ct, dst, wb, srcb, nkt, bias_sb, c0, c1):
                """dst[:, c0:c1] = (W @ src)[ct*P:(ct+1)*P, c0:c1] + bias."""
                ps = psQK.tile([P, 512], F32, name="ps", tag="psqk")
                for kt in range(nkt):
                    nc.tensor.matmul(
                        ps[:, 0:c1 - c0],
                        wb[:, kt, ct * P:(ct + 1) * P],
                        srcb[:, kt, c0:c1],
                        start=(kt == 0),
                        stop=(kt == nkt - 1),
                    )
                nc.vector.tensor_scalar_add(
                    dst[:, c0:c1], ps[:, 0:c1 - c0], bias_sb[:, ct:ct + 1]
                )

            # ---- VT projection: VT[n, c] = sum_k y[k, n] * wvT[k, c]  (+ bias row)
            # medium priority (above AV/out-proj, below the exp chain): every
            # AV cell reads all eight vt tiles, so VT completing late gates
            # the whole AV pipeline. Groups alternate two pools so the
            # slot-release chain (MMs -> DVE copy -> next group) overlaps.
            def vt_proj(nt):
                with tc.high_priority(offset=HIPRI // 2):
                    # psAV pool: idle until the first AV group (~35us), so VT
                    # double-buffers there without touching the exp-critical
                    # psQK pool
                    ps = psAV.tile([P, 512], F32, name="ps", tag="av")
                    for kt in range(CCT):
                        nc.tensor.matmul(
                            ps[:],
                            yb[:, kt, nt * P:(nt + 1) * P],
                            wvb[:, kt, :],
                            start=(kt == 0),
                            stop=False,
                        )
                    nc.tensor.matmul(ps[:], ones_r[:], bv_sb[:], start=False, stop=True)
                    # scatter into [h][0:64] slots (col h*65+64 stays 1.0)
                    nc.vector.tensor_copy(
                        out=vt_sb[nt].rearrange("p (h e) -> p h e", e=DH + 1)[:, :, 0:DH],
                        in_=ps.rearrange("p (h d) -> p h d", d=DH),
                    )

            # pair-0 prologue: K in three column chunks chasing the y DMAs,
            # so exp#1 only waits on y strip0
            with tc.high_priority(offset=HIPRI):
                qk_proj_cols(0, q_sb[0], wqb, xb, CT, bq_sb, 0, 512)
                qk_proj_cols(0, k_sb[0], wkb, yb, CCT, bk_sb, 0, P)
                qk_proj_cols(0, k_sb[0], wkb, yb, CCT, bk_sb, P, 512)
                qk_proj_cols(0, k_sb[0], wkb, yb, CCT, bk_sb, 512, 768)
                qk_proj_cols(0, q_sb[0], wqb, xb, CT, bq_sb, 512, N)
                qk_proj_cols(0, k_sb[0], wkb, yb, CCT, bk_sb, 768, N)

            # ---- attention ----
            qk_sb = [bigp.tile([P, N], BF16, name=f"qk{t}", tag=f"qk{t}")
                     for t in range(CT)]

            def ecol(hh, qt):
                return (qt // 4) * 1024 + hh * 512 + (qt % 4) * P

            def emit_av_bank(ph, hh, ets, bank, quirk_spread):
                """AV for one (head, 4-qt bank). Cell accumulation groups within
                one PSUM bank must be sequential (start=True clears has_written
                for the whole bank), so cells run kt-inner back-to-back."""
                h = 2 * ph + hh
                oa = oap.tile([P, 256], BF16, name="oa", tag="oa")
                av = psAV.tile([P, 512], F32, name="av", tag="av")
                for qq in range(4):
                    qt = bank * 4 + qq
                    for kt in range(QT):
                        nc.tensor.matmul(
                            av[:, qq * P:qq * P + DH + 1],
                            ets[kt][:, ecol(hh, qt):ecol(hh, qt) + P],
                            vt_sb[kt][:, h * (DH + 1):(h + 1) * (DH + 1)],
                            start=(kt == 0),
                            stop=(kt == QT - 1),
                        )
                # batched normalize: one reciprocal for the bank's 4
                # denominators, then per-cell scale + shuffle. Medium
                # priority: the psAV slot only frees after these reads, so
                # they pace the whole AV chain.
                with tc.high_priority(offset=HIPRI // 2):
                    rcp = rcpp.tile([P, 4], F32, name="rcp", tag="rcp")
                    nc.vector.reciprocal(
                        rcp[:], av.rearrange("p (q c) -> p q c", c=P)[:, :, DH]
                    )
                    for qq in range(4):
                        nc.vector.tensor_scalar_mul(
                            oa[:, qq * DH:(qq + 1) * DH],
                            av[:, qq * P:qq * P + DH], rcp[:, qq:qq + 1],
                        )
                # quirk shuffle:
                # qk[ph][hh*64 + qt*8 + p//16, 64*(p%16)+d] = O_h[qt*128+p, d]
                for qq in range(4):
                    qt = bank * 4 + qq
                    eng = nc.scalar if (quirk_spread and qq % 2 == 1) else nc.sync
                    eng.dma_start(
                        qk_sb[ph][hh * 64 + qt * 8: hh * 64 + qt * 8 + 8, :],
                        oa[:, qq * DH:(qq + 1) * DH],
                    )

            for ph in range(NH // 2):
                ets = [etp.tile([P, 2048], name="et", tag="et", dtype=BF16)
                       for _ in range(QT)]
                last_pair = ph == NH // 2 - 1
                for half in range(2):
                    for kt in range(QT):
                        with tc.high_priority(offset=HIPRI):
                            sps = psS.tile([P, 1024], F32, name="sps", tag="pss")
                            for hh in range(2):
                                bp = hh * DH
                                nc.tensor.matmul(
                                    sps[:, hh * 512:(hh + 1) * 512],
                                    k_sb[ph][bp:bp + DH, kt * P:(kt + 1) * P],
                                    q_sb[ph][bp:bp + DH, half * 512:(half + 1) * 512],
                                    start=True,
                                    stop=True,
                                )
                            nc.scalar.activation(
                                ets[kt][:, half * 1024:(half + 1) * 1024], sps[:],
                                EXP, scale=0.125,
                            )
                        if ph == 0 and half == 0:
                            vt_proj(kt)  # before first reader (AV below)
                        # next-pair projections gate the next pair's exp chain
                        if ph + 1 < NH // 2:
                            np1 = ph + 1
                            u = half * 8 + kt
                            if u == 1:
                                with tc.high_priority(offset=HIPRI):
                                    qk_proj_cols(np1, q_sb[np1], wqb, xb, CT, bq_sb, 0, 512)
                            elif u == 3:
                                with tc.high_priority(offset=HIPRI):
                                    qk_proj_cols(np1, k_sb[np1], wkb, yb, CCT, bk_sb, 0, 512)
                            elif u == 9:
                                with tc.high_priority(offset=HIPRI):
                                    qk_proj_cols(np1, q_sb[np1], wqb, xb, CT, bq_sb, 512, N)
                            elif u == 11:
                                with tc.high_priority(offset=HIPRI):
                                    qk_proj_cols(np1, k_sb[np1], wkb, yb, CCT, bk_sb, 512, N)
                    # bank `half` only needs the ETs of this half: emit right
                    # after the half's last unit so it overlaps the other half
                    emit_av_bank(ph, 0, ets, half, quirk_spread=last_pair and half == 1)
                    emit_av_bank(ph, 1, ets, half, quirk_spread=last_pair and half == 1)

            # ---- output projection ----
            # alternate groups between psQK (free during pair 3) and psS
            # (free after the last exps) so consecutive groups pipeline
            # PE-matmul / DVE-bias / DMA instead of serializing on one slot
            out3 = out.rearrange("(t p) n -> p t n", p=P)
            groups = [(ct, half) for half in range(2) for ct in range(CT)]
            for gi, (ct, half) in enumerate(groups):
                pool, tag = [(psQK, "psqk"), (psS, "pss")][gi % 2]
                ps = pool.tile([P, 512], F32, name="ps", tag=tag)
                for kt in range(CT):
                    nc.tensor.matmul(
                        ps[:, 0:512],
                        wob[:, kt, ct * P:(ct + 1) * P],
                        qk_sb[kt][:, half * 512:(half + 1) * 512],
                        start=(kt == 0),
                        stop=(kt == CT - 1),
                    )
                ev = evp.tile([P, 512], BF16, name="ev", tag="ev")
                nc.vector.tensor_scalar_add(ev[:], ps[:, 0:512], bo_sb[:, ct:ct + 1])
                eng = nc.scalar if gi % 2 == 1 else nc.sync
                eng.dma_start(out3[:, ct, half * 512:(half + 1) * 512], ev[:])

    nc.compile()
    return nc


def kernel(**inputs) -> np.ndarray:
    bf = ml_dtypes.bfloat16
    x = np.ascontiguousarray(np.asarray(inputs["x"], dtype=np.float32).astype(bf))
    y = np.ascontiguousarray(np.asarray(inputs["y"], dtype=np.float32).astype(bf))
    wqT = np.ascontiguousarray(np.asarray(inputs["w_q"], dtype=np.float32).T.astype(bf))
    wkT = np.ascontiguousarray(np.asarray(inputs["w_k"], dtype=np.float32).T.astype(bf))
    wvT = np.ascontiguousarray(np.asarray(inputs["w_v"], dtype=np.float32).T.astype(bf))
    woT = np.ascontiguousarray(np.asarray(inputs["w_o"], dtype=np.float32).T.astype(bf))
    bq = np.ascontiguousarray(np.asarray(inputs["b_q"], dtype=np.float32))
    bk = np.ascontiguousarray(np.asarray(inputs["b_k"], dtype=np.float32))
    bv = np.ascontiguousarray(np.asarray(inputs["b_v"], dtype=np.float32).astype(bf))
    bo = np.ascontiguousarray(np.asarray(inputs["b_o"], dtype=np.float32))

    if "nc" not in _CACHE:
        _CACHE["nc"] = _build_nc()
    nc = _CACHE["nc"]

    in_maps = []
    for b in range(B):
        in_maps.append({
            "x": np.ascontiguousarray(x[b].reshape(C, N)),
            "y": np.ascontiguousarray(y[b].reshape(CC, N)),
            "wqT": wqT, "wkT": wkT, "wvT": wvT, "woT": woT,
            "bq": bq, "bk": bk, "bv": bv, "bo": bo,
        })
    res = run_bass_kernel_spmd(nc, in_maps, core_ids=list(range(B)))
    return np.stack([
        np.asarray(res.results[b]["out"]).astype(np.float32).reshape(C, HW, HW)
        for b in range(B)
    ])


# revision 28
# speedup vs baseline: 1.2360x; 1.0529x over previous
"""CrossAttention2D Trainium2 kernel (v4).

Sharding: data-parallel over batch. B=8 -> one batch element per NeuronCore,
no collectives. Weights replicated; host pre-transposes and casts to bf16.

Per-core math (C=512, Ccross=768, N=1024, 8 heads x 64):
  Q = Wq @ x_b          [C, N]   bf16
  K = Wk @ y_b          [C, N]   bf16
  VTa = [(Wv @ y_b).T | 1]       [N, 8*(64+1)] bf16 (ones col per head)
  per head pair ph (heads at PE rows 0/64, row-tiled scores):
    S[k, q] = K_h^T Q_h          psum [128, 1024] per (half, kt), ping-pong
    ET = exp(S/8)                ACT -> SBUF bf16, resident for whole pair
    O_aug[q, 0:65] = ET_tile^T @ VTa_h   (ET stationary, 65-col streams,
                                          kt-inner per cell; col 64 = denom)
    O = O_aug[:, :64] / O_aug[:, 64]     (DVE reciprocal + scale) -> bf16
  quirk: out_flat[h*64 + q//16, 64*(q%16) + d] = O_h[q, d]  (DMA shuffle)
  out = Wo @ quirk + bo          [C, N] -> bf16 out, host casts to f32

v4 scheduling (from v2/v3 ntff traces):
  - separate PSUM pools for next-pair Q/K proj (psQK) vs VT proj (psVT):
    in v2/v3 they shared one 2-slot pool, so the exp-critical Q/K proj
    serialized behind low-priority VT work -> 5-14us ACT stall per pair
  - exp-critical chain (Q/K proj, scores, exp) under tc.high_priority
  - consolidated 3D input DMAs (one instr per tensor slice group, ~600ns
    issue each) ordered so exp#1 only waits on ~1MB: wq-ct0, x-h0,
    wk-ct0, y-strip0; K proj for pair 0 follows the slices
  - AV emitted per (pair, bank): bank0 only needs half0 ETs
  - last pair's quirk + output DMAs alternate sync/scalar (ACT idle)
  - out-proj groups ct0/ct1 use psQK/psVT (free during pairs 1-3 -> they
    pre-run), ct2/ct3 use psS (free after the last exp)
"""

import numpy as np
import ml_dtypes

import concourse.bass as bass
import concourse.mybir as mybir
import concourse.tile as tile
from concourse import bacc
from concourse.bass_utils import run_bass_kernel_spmd

P = 128
C = 512          # d_embed
CC = 768         # d_cross
N = 1024         # H*W = 32*32
NH = 8
DH = 64
CT = C // P      # 4
CCT = CC // P    # 6
QT = N // P      # 8
HW = 32
B = 8
F32 = mybir.dt.float32
BF16 = mybir.dt.bfloat16
HIPRI = 1_000_000

_CACHE = {}


def _build_nc():
    nc = bacc.Bacc("TRN2", target_bir_lowering=False, debug=False, num_devices=B)

    x = nc.dram_tensor("x", [C, N], BF16, kind="ExternalInput")
    y = nc.dram_tensor("y", [CC, N], BF16, kind="ExternalInput")
    wqT = nc.dram_tensor("wqT", [C, C], BF16, kind="ExternalInput")
    wkT = nc.dram_tensor("wkT", [CC, C], BF16, kind="ExternalInput")
    wvT = nc.dram_tensor("wvT", [CC, C], BF16, kind="ExternalInput")
    woT = nc.dram_tensor("woT", [C, C], BF16, kind="ExternalInput")
    bq = nc.dram_tensor("bq", [C], F32, kind="ExternalInput")
    bk = nc.dram_tensor("bk", [C], F32, kind="ExternalInput")
    bv = nc.dram_tensor("bv", [C], BF16, kind="ExternalInput")
    bo = nc.dram_tensor("bo", [C], F32, kind="ExternalInput")
    out = nc.dram_tensor("out", [C, N], BF16, kind="ExternalOutput")

    EXP = mybir.ActivationFunctionType.Exp

    with tile.TileContext(nc) as tc:
        with (
            tc.tile_pool(name="const", bufs=1) as constp,
            tc.tile_pool(name="big", bufs=1) as bigp,
            tc.tile_pool(name="et", bufs=24) as etp,
            tc.tile_pool(name="oa", bufs=8) as oap,
            tc.tile_pool(name="ev", bufs=3) as evp,
            tc.tile_pool(name="rcp", bufs=4) as rcpp,
            tc.tile_pool(name="psS", bufs=2, space="PSUM") as psS,
            tc.tile_pool(name="psAV", bufs=3, space="PSUM") as psAV,
            tc.tile_pool(name="psQK", bufs=1, space="PSUM") as psQK,
        ):
            # ---- constants ----
            ones_r = constp.tile([1, P], BF16, name="ones_r", tag="ones_r")
            nc.vector.memset(ones_r[:], 1.0)
            # preload the exp table set early so the ~2.7us ACT_TABLE_LOAD
            # overlaps the input DMA phase instead of the first real exp
            dmy = constp.tile([P, 1], F32, name="dmy", tag="dmy")
            nc.vector.memset(dmy[:], 0.0)
            dmy2 = constp.tile([P, 1], F32, name="dmy2", tag="dmy2")
            nc.scalar.activation(dmy2[:], dmy[:], EXP)

            bq_sb = constp.tile([P, CT], F32, name="bq", tag="bq")
            bk_sb = constp.tile([P, CT], F32, name="bk", tag="bk")
            bo_sb = constp.tile([P, CT], F32, name="bo", tag="bo")
            bv_sb = constp.tile([1, C], BF16, name="bv", tag="bv")

            # ---- consolidated input tiles (3D views) ----
            x3 = x.rearrange("(t p) n -> p t n", p=P)
            y3 = y.rearrange("(t p) n -> p t n", p=P)
            wq3 = wqT.rearrange("(t p) m -> p t m", p=P)
            wk3 = wkT.rearrange("(t p) m -> p t m", p=P)
            wv3 = wvT.rearrange("(t p) m -> p t m", p=P)
            wo3 = woT.rearrange("(t p) m -> p t m", p=P)

            xb = bigp.tile([P, CT, N], BF16, name="xb", tag="xb")
            yb = bigp.tile([P, CCT, N], BF16, name="yb", tag="yb")
            wqb = bigp.tile([P, CT, C], BF16, name="wqb", tag="wqb")
            wkb = bigp.tile([P, CCT, C], BF16, name="wkb", tag="wkb")
            wvb = bigp.tile([P, CCT, C], BF16, name="wvb", tag="wvb")
            wob = bigp.tile([P, CT, C], BF16, name="wob", tag="wob")

            # all input DMAs on ONE queue (sync) in strict priority order:
            # the DGE issue order is also the HBM bandwidth allocation order,
            # so eager second-wave DMAs must not race the exp#1-critical wave
            # (v4 lost ~5us to x-h0 sharing bandwidth with later loads)
            nc.scalar.dma_start(bq_sb[:], bq.rearrange("(o p) -> p o", p=P))
            nc.scalar.dma_start(bk_sb[:], bk.rearrange("(o p) -> p o", p=P))
            nc.sync.dma_start(wqb[:, :, 0:P], wq3[:, :, 0:P])
            nc.sync.dma_start(xb[:, :, 0:512], x3[:, :, 0:512])
            nc.sync.dma_start(wkb[:, :, 0:P], wk3[:, :, 0:P])
            nc.sync.dma_start(yb[:, :, 0:P], y3[:, :, 0:P])
            # second wave, in order of first use (wv early: the VT chain
            # gates the whole AV pipeline; y half1 split so K cols 512-767
            # are projectable sooner)
            nc.sync.dma_start(yb[:, :, P:512], y3[:, :, P:512])
            nc.sync.dma_start(yb[:, :, 512:768], y3[:, :, 512:768])
            nc.sync.dma_start(yb[:, :, 768:N], y3[:, :, 768:N])
            nc.sync.dma_start(wvb[:], wv3[:])
            nc.sync.dma_start(xb[:, :, 512:N], x3[:, :, 512:N])
            nc.sync.dma_start(wqb[:, :, P:C], wq3[:, :, P:C])
            nc.sync.dma_start(wkb[:, :, P:C], wk3[:, :, P:C])
            nc.sync.dma_start(wob[:], wo3[:])
            nc.sync.dma_start(bo_sb[:], bo.rearrange("(o p) -> p o", p=P))
            nc.sync.dma_start(bv_sb[:], bv[None, :])

            q_sb = [bigp.tile([P, N], BF16, name=f"q{t}", tag=f"q{t}") for t in range(CT)]
            k_sb = [bigp.tile([P, N], BF16, name=f"k{t}", tag=f"k{t}") for t in range(CT)]
            # VTa buffer: per n-tile, cols laid out [h][65] with col h*65+64 == 1.0
            vt_sb = [bigp.tile([P, NH * (DH + 1)], BF16, name=f"vt{t}", tag=f"vt{t}")
                     for t in range(QT)]
            for t in range(QT):
                nc.gpsimd.memset(vt_sb[t][:], 1.0)

            # PE warm-up: dummy matmuls while the input DMAs stream, so the
            # HAM clock gate is at 8/8 before the first projection and exp#1
            # isn't paying cold-clock prices (~3.4us of sustained PE activity
            # flips the gate)
            wup = psAV.tile([P, 512], F32, name="wup", tag="av")
            for i in range(18):
                nc.tensor.matmul(
                    wup[:], vt_sb[6][:, 0:P], vt_sb[7][:, 0:512],
                    start=True, stop=True,
                )

            # ---- projection helpers ----
            def qk_proj_cols(ct, dst, wb, srcb, nkt, bias_sb, c0, c1):
                """dst[:, c0:c1] = (W @ src)[ct*P:(ct+1)*P, c0:c1] + bias."""
                ps = psQK.tile([P, 512], F32, name="ps", tag="psqk")
                for kt in range(nkt):
                    nc.tensor.matmul(
                        ps[:, 0:c1 - c0],
                        wb[:, kt, ct * P:(ct + 1) * P],
                        srcb[:, kt, c0:c1],
                        start=(kt == 0),
                        stop=(kt == nkt - 1),
                    )
                nc.vector.tensor_scalar_add(
                    dst[:, c0:c1], ps[:, 0:c1 - c0], bias_sb[:, ct:ct + 1]
                )

            # ---- VT projection: VT[n, c] = sum_k y[k, n] * wvT[k, c]  (+ bias row)
            # medium priority (above AV/out-proj, below the exp chain): every
            # AV cell reads all eight vt tiles, so VT completing late gates
            # the whole AV pipeline. Groups alternate two pools so the
            # slot-release chain (MMs -> DVE copy -> next group) overlaps.
            def vt_proj(nt):
                with tc.high_priority(offset=HIPRI // 2):
                    # psAV pool: idle until the first AV group (~35us), so VT
                    # double-buffers there without touching the exp-critical
                    # psQK pool
                    ps = psAV.tile([P, 512], F32, name="ps", tag="av")
                    for kt in range(CCT):
                        nc.tensor.matmul(
                            ps[:],
                            yb[:, kt, nt * P:(nt + 1) * P],
                            wvb[:, kt, :],
                            start=(kt == 0),
                            stop=False,
                        )
                    nc.tensor.matmul(ps[:], ones_r[:], bv_sb[:], start=False, stop=True)
                    # scatter into [h][0:64] slots (col h*65+64 stays 1.0)
                    nc.vector.tensor_copy(
                        out=vt_sb[nt].rearrange("p (h e) -> p h e", e=DH + 1)[:, :, 0:DH],
                        in_=ps.rearrange("p (h d) -> p h d", d=DH),
                    )

            # pair-0 prologue: K in three column chunks chasing the y DMAs,
            # so exp#1 only waits on y strip0
            with tc.high_priority(offset=HIPRI):
                qk_proj_cols(0, q_sb[0], wqb, xb, CT, bq_sb, 0, 512)
                qk_proj_cols(0, k_sb[0], wkb, yb, CCT, bk_sb, 0, P)
                qk_proj_cols(0, k_sb[0], wkb, yb, CCT, bk_sb, P, 512)
                qk_proj_cols(0, k_sb[0], wkb, yb, CCT, bk_sb, 512, 768)
                qk_proj_cols(0, q_sb[0], wqb, xb, CT, bq_sb, 512, N)
                qk_proj_cols(0, k_sb[0], wkb, yb, CCT, bk_sb, 768, N)

            # ---- attention ----
            qk_sb = [bigp.tile([P, N], BF16, name=f"qk{t}", tag=f"qk{t}")
                     for t in range(CT)]

            def ecol(hh, qt):
                return (qt // 4) * 1024 + hh * 512 + (qt % 4) * P

            def emit_av_bank(ph, hh, ets, bank, quirk_spread):
                """AV for one (head, 4-qt bank). Cell accumulation groups within
                one PSUM bank must be sequential (start=True clears has_written
                for the whole bank), so cells run kt-inner back-to-back."""
                h = 2 * ph + hh
                oa = oap.tile([P, 256], BF16, name="oa", tag="oa")
                with tc.high_priority(offset=HIPRI // 2):
                    av = psAV.tile([P, 512], F32, name="av", tag="av")
                    for qq in range(4):
                        qt = bank * 4 + qq
                        for kt in range(QT):
                            nc.tensor.matmul(
                                av[:, qq * P:qq * P + DH + 1],
                                ets[kt][:, ecol(hh, qt):ecol(hh, qt) + P],
                                vt_sb[kt][:, h * (DH + 1):(h + 1) * (DH + 1)],
                                start=(kt == 0),
                                stop=(kt == QT - 1),
                            )
                # batched normalize: one reciprocal for the bank's 4
                # denominators, then per-cell scale + shuffle. Medium
                # priority: the psAV slot only frees after these reads, so
                # they pace the whole AV chain.
                with tc.high_priority(offset=HIPRI // 2):
                    rcp = rcpp.tile([P, 4], F32, name="rcp", tag="rcp")
                    nc.vector.reciprocal(
                        rcp[:], av.rearrange("p (q c) -> p q c", c=P)[:, :, DH]
                    )
                    for qq in range(4):
                        nc.vector.tensor_scalar_mul(
                            oa[:, qq * DH:(qq + 1) * DH],
                            av[:, qq * P:qq * P + DH], rcp[:, qq:qq + 1],
                        )
                # quirk shuffle:
                # qk[ph][hh*64 + qt*8 + p//16, 64*(p%16)+d] = O_h[qt*128+p, d]
                # later pairs alternate onto the scalar queue (its exp stream
                # is ending; the sync queue otherwise becomes the AV pacer)
                for qq in range(4):
                    qt = bank * 4 + qq
                    eng = nc.scalar if (quirk_spread and qq % 2 == 1) else nc.sync
                    eng.dma_start(
                        qk_sb[ph][hh * 64 + qt * 8: hh * 64 + qt * 8 + 8, :],
                        oa[:, qq * DH:(qq + 1) * DH],
                    )

            for ph in range(NH // 2):
                ets = [etp.tile([P, 2048], name="et", tag="et", dtype=BF16)
                       for _ in range(QT)]
                last_pair = ph == NH // 2 - 1
                for half in range(2):
                    for kt in range(QT):
                        with tc.high_priority(offset=HIPRI):
                            sps = psS.tile([P, 1024], F32, name="sps", tag="pss")
                            for hh in range(2):
                                bp = hh * DH
                                nc.tensor.matmul(
                                    sps[:, hh * 512:(hh + 1) * 512],
                                    k_sb[ph][bp:bp + DH, kt * P:(kt + 1) * P],
                                    q_sb[ph][bp:bp + DH, half * 512:(half + 1) * 512],
                                    start=True,
                                    stop=True,
                                )
                            nc.scalar.activation(
                                ets[kt][:, half * 1024:(half + 1) * 1024], sps[:],
                                EXP, scale=0.125,
                            )
                        if ph == 0 and half == 0:
                            vt_proj(kt)  # before first reader (AV below)
                        # next-pair projections gate the next pair's exp chain
                        if ph + 1 < NH // 2:
                            np1 = ph + 1
                            u = half * 8 + kt
                            if u == 1:
                                with tc.high_priority(offset=HIPRI):
                                    qk_proj_cols(np1, q_sb[np1], wqb, xb, CT, bq_sb, 0, 512)
                            elif u == 3:
                                with tc.high_priority(offset=HIPRI):
                                    qk_proj_cols(np1, k_sb[np1], wkb, yb, CCT, bk_sb, 0, 512)
                            elif u == 9:
                                with tc.high_priority(offset=HIPRI):
                                    qk_proj_cols(np1, q_sb[np1], wqb, xb, CT, bq_sb, 512, N)
                            elif u == 11:
                                with tc.high_priority(offset=HIPRI):
                                    qk_proj_cols(np1, k_sb[np1], wkb, yb, CCT, bk_sb, 512, N)
                    # bank `half` only needs the ETs of this half: emit right
                    # after the half's last unit so it overlaps the other half
                    spread = ph >= 2
                    emit_av_bank(ph, 0, ets, half, quirk_spread=spread)
                    emit_av_bank(ph, 1, ets, half, quirk_spread=spread)

            # ---- output projection ----
            # alternate groups between psQK (free during pair 3) and psS
            # (free after the last exps) so consecutive groups pipeline
            # PE-matmul / DVE-bias / DMA instead of serializing on one slot
            out3 = out.rearrange("(t p) n -> p t n", p=P)
            groups = [(ct, half) for half in range(2) for ct in range(CT)]
            for gi, (ct, half) in enumerate(groups):
                pool, tag = [(psQK, "psqk"), (psS, "pss")][gi % 2]
                ps = pool.tile([P, 512], F32, name="ps", tag=tag)
                for kt in range(CT):
                    nc.tensor.matmul(
                        ps[:, 0:512],
                        wob[:, kt, ct * P:(ct + 1) * P],
                        qk_sb[kt][:, half * 512:(half + 1) * 512],
                        start=(kt == 0),
                        stop=(kt == CT - 1),
                    )
                ev = evp.tile([P, 512], BF16, name="ev", tag="ev")
                nc.vector.tensor_scalar_add(ev[:], ps[:, 0:512], bo_sb[:, ct:ct + 1])
                eng = nc.scalar if gi % 2 == 1 else nc.sync
                eng.dma_start(out3[:, ct, half * 512:(half + 1) * 512], ev[:])

    nc.compile()
    return nc


def kernel(**inputs) -> np.ndarray:
    bf = ml_dtypes.bfloat16
    x = np.ascontiguousarray(np.asarray(inputs["x"], dtype=np.float32).astype(bf))
    y = np.ascontiguousarray(np.asarray(inputs["y"], dtype=np.float32).astype(bf))
    wqT = np.ascontiguousarray(np.asarray(inputs["w_q"], dtype=np.float32).T.astype(bf))
    wkT = np.ascontiguousarray(np.asarray(inputs["w_k"], dtype=np.float32).T.astype(bf))
    wvT = np.ascontiguousarray(np.asarray(inputs["w_v"], dtype=np.float32).T.astype(bf))
    woT = np.ascontiguousarray(np.asarray(inputs["w_o"], dtype=np.float32).T.astype(bf))
    bq = np.ascontiguousarray(np.asarray(inputs["b_q"], dtype=np.float32))
    bk = np.ascontiguousarray(np.asarray(inputs["b_k"], dtype=np.float32))
    bv = np.ascontiguousarray(np.asarray(inputs["b_v"], dtype=np.float32).astype(bf))
    bo = np.ascontiguousarray(np.asarray(inputs["b_o"], dtype=np.float32))

    if "nc" not in _CACHE:
        _CACHE["nc"] = _build_nc()
    nc = _CACHE["nc"]

    in_maps = []
    for b in range(B):
        in_maps.append({
            "x": np.ascontiguousarray(x[b].reshape(C, N)),
            "y": np.ascontiguousarray(y[b].reshape(CC, N)),
            "wqT": wqT, "wkT": wkT, "wvT": wvT, "woT": woT,
            "bq": bq, "bk": bk, "bv": bv, "bo": bo,
        })
    res = run_bass_kernel_spmd(nc, in_maps, core_ids=list(range(B)))
    return np.stack([
        np.asarray(res.results[b]["out"]).astype(np.float32).reshape(C, HW, HW)
        for b in range(B)
    ])


# revision 30
# speedup vs baseline: 1.2434x; 1.0060x over previous
"""CrossAttention2D Trainium2 kernel (v4).

Sharding: data-parallel over batch. B=8 -> one batch element per NeuronCore,
no collectives. Weights replicated; host pre-transposes and casts to bf16.

Per-core math (C=512, Ccross=768, N=1024, 8 heads x 64):
  Q = Wq @ x_b          [C, N]   bf16
  K = Wk @ y_b          [C, N]   bf16
  VTa = [(Wv @ y_b).T | 1]       [N, 8*(64+1)] bf16 (ones col per head)
  per head pair ph (heads at PE rows 0/64, row-tiled scores):
    S[k, q] = K_h^T Q_h          psum [128, 1024] per (half, kt), ping-pong
    ET = exp(S/8)                ACT -> SBUF bf16, resident for whole pair
    O_aug[q, 0:65] = ET_tile^T @ VTa_h   (ET stationary, 65-col streams,
                                          kt-inner per cell; col 64 = denom)
    O = O_aug[:, :64] / O_aug[:, 64]     (DVE reciprocal + scale) -> bf16
  quirk: out_flat[h*64 + q//16, 64*(q%16) + d] = O_h[q, d]  (DMA shuffle)
  out = Wo @ quirk + bo          [C, N] -> bf16 out, host casts to f32

v4 scheduling (from v2/v3 ntff traces):
  - separate PSUM pools for next-pair Q/K proj (psQK) vs VT proj (psVT):
    in v2/v3 they shared one 2-slot pool, so the exp-critical Q/K proj
    serialized behind low-priority VT work -> 5-14us ACT stall per pair
  - exp-critical chain (Q/K proj, scores, exp) under tc.high_priority
  - consolidated 3D input DMAs (one instr per tensor slice group, ~600ns
    issue each) ordered so exp#1 only waits on ~1MB: wq-ct0, x-h0,
    wk-ct0, y-strip0; K proj for pair 0 follows the slices
  - AV emitted per (pair, bank): bank0 only needs half0 ETs
  - last pair's quirk + output DMAs alternate sync/scalar (ACT idle)
  - out-proj groups ct0/ct1 use psQK/psVT (free during pairs 1-3 -> they
    pre-run), ct2/ct3 use psS (free after the last exp)
"""

import numpy as np
import ml_dtypes

import concourse.bass as bass
import concourse.mybir as mybir
import concourse.tile as tile
from concourse import bacc
from concourse.bass_utils import run_bass_kernel_spmd

P = 128
C = 512          # d_embed
CC = 768         # d_cross
N = 1024         # H*W = 32*32
NH = 8
DH = 64
CT = C // P      # 4
CCT = CC // P    # 6
QT = N // P      # 8
HW = 32
B = 8
F32 = mybir.dt.float32
BF16 = mybir.dt.bfloat16
HIPRI = 1_000_000

_CACHE = {}


def _build_nc():
    nc = bacc.Bacc("TRN2", target_bir_lowering=False, debug=False, num_devices=B)

    x = nc.dram_tensor("x", [C, N], BF16, kind="ExternalInput")
    y = nc.dram_tensor("y", [CC, N], BF16, kind="ExternalInput")
    wqT = nc.dram_tensor("wqT", [C, C], BF16, kind="ExternalInput")
    wkT = nc.dram_tensor("wkT", [CC, C], BF16, kind="ExternalInput")
    wvT = nc.dram_tensor("wvT", [CC, C], BF16, kind="ExternalInput")
    woT = nc.dram_tensor("woT", [C, C], BF16, kind="ExternalInput")
    bq = nc.dram_tensor("bq", [C], F32, kind="ExternalInput")
    bk = nc.dram_tensor("bk", [C], F32, kind="ExternalInput")
    bv = nc.dram_tensor("bv", [C], BF16, kind="ExternalInput")
    bo = nc.dram_tensor("bo", [C], F32, kind="ExternalInput")
    out = nc.dram_tensor("out", [C, N], BF16, kind="ExternalOutput")

    EXP = mybir.ActivationFunctionType.Exp

    with tile.TileContext(nc) as tc:
        with (
            tc.tile_pool(name="const", bufs=1) as constp,
            tc.tile_pool(name="big", bufs=1) as bigp,
            tc.tile_pool(name="et", bufs=24) as etp,
            tc.tile_pool(name="oa", bufs=8) as oap,
            tc.tile_pool(name="ev", bufs=3) as evp,
            tc.tile_pool(name="rcp", bufs=4) as rcpp,
            tc.tile_pool(name="psS", bufs=2, space="PSUM") as psS,
            tc.tile_pool(name="psAV", bufs=3, space="PSUM") as psAV,
            tc.tile_pool(name="psQK", bufs=1, space="PSUM") as psQK,
        ):
            # ---- constants ----
            ones_r = constp.tile([1, P], BF16, name="ones_r", tag="ones_r")
            nc.vector.memset(ones_r[:], 1.0)
            # preload the exp table set early so the ~2.7us ACT_TABLE_LOAD
            # overlaps the input DMA phase instead of the first real exp
            dmy = constp.tile([P, 1], F32, name="dmy", tag="dmy")
            nc.vector.memset(dmy[:], 0.0)
            dmy2 = constp.tile([P, 1], F32, name="dmy2", tag="dmy2")
            nc.scalar.activation(dmy2[:], dmy[:], EXP)

            bq_sb = constp.tile([P, CT], F32, name="bq", tag="bq")
            bk_sb = constp.tile([P, CT], F32, name="bk", tag="bk")
            bo_sb = constp.tile([P, CT], F32, name="bo", tag="bo")
            bv_sb = constp.tile([1, C], BF16, name="bv", tag="bv")

            # ---- consolidated input tiles (3D views) ----
            x3 = x.rearrange("(t p) n -> p t n", p=P)
            y3 = y.rearrange("(t p) n -> p t n", p=P)
            wq3 = wqT.rearrange("(t p) m -> p t m", p=P)
            wk3 = wkT.rearrange("(t p) m -> p t m", p=P)
            wv3 = wvT.rearrange("(t p) m -> p t m", p=P)
            wo3 = woT.rearrange("(t p) m -> p t m", p=P)

            xb = bigp.tile([P, CT, N], BF16, name="xb", tag="xb")
            yb = bigp.tile([P, CCT, N], BF16, name="yb", tag="yb")
            wqb = bigp.tile([P, CT, C], BF16, name="wqb", tag="wqb")
            wkb = bigp.tile([P, CCT, C], BF16, name="wkb", tag="wkb")
            wvb = bigp.tile([P, CCT, C], BF16, name="wvb", tag="wvb")
            wob = bigp.tile([P, CT, C], BF16, name="wob", tag="wob")

            # all input DMAs on ONE queue (sync) in strict priority order:
            # the DGE issue order is also the HBM bandwidth allocation order,
            # so eager second-wave DMAs must not race the exp#1-critical wave
            # (v4 lost ~5us to x-h0 sharing bandwidth with later loads)
            nc.scalar.dma_start(bq_sb[:], bq.rearrange("(o p) -> p o", p=P))
            nc.scalar.dma_start(bk_sb[:], bk.rearrange("(o p) -> p o", p=P))
            nc.sync.dma_start(wqb[:, :, 0:P], wq3[:, :, 0:P])
            nc.sync.dma_start(xb[:, :, 0:512], x3[:, :, 0:512])
            nc.sync.dma_start(wkb[:, :, 0:P], wk3[:, :, 0:P])
            nc.sync.dma_start(yb[:, :, 0:P], y3[:, :, 0:P])
            # second wave, in order of first use (wv early: the VT chain
            # gates the whole AV pipeline; y half1 split so K cols 512-767
            # are projectable sooner)
            nc.sync.dma_start(yb[:, :, P:512], y3[:, :, P:512])
            nc.sync.dma_start(yb[:, :, 512:768], y3[:, :, 512:768])
            nc.sync.dma_start(yb[:, :, 768:N], y3[:, :, 768:N])
            nc.sync.dma_start(wvb[:], wv3[:])
            nc.sync.dma_start(xb[:, :, 512:N], x3[:, :, 512:N])
            nc.sync.dma_start(wqb[:, :, P:C], wq3[:, :, P:C])
            nc.sync.dma_start(wkb[:, :, P:C], wk3[:, :, P:C])
            nc.sync.dma_start(wob[:], wo3[:])
            nc.sync.dma_start(bo_sb[:], bo.rearrange("(o p) -> p o", p=P))
            nc.sync.dma_start(bv_sb[:], bv[None, :])

            q_sb = [bigp.tile([P, N], BF16, name=f"q{t}", tag=f"q{t}") for t in range(CT)]
            k_sb = [bigp.tile([P, N], BF16, name=f"k{t}", tag=f"k{t}") for t in range(CT)]
            # VTa buffer: per n-tile, cols laid out [h][65] with col h*65+64 == 1.0
            vt_sb = [bigp.tile([P, NH * (DH + 1)], BF16, name=f"vt{t}", tag=f"vt{t}")
                     for t in range(QT)]
            for t in range(QT):
                nc.gpsimd.memset(vt_sb[t][:], 1.0)

            # PE warm-up: dummy matmuls while the input DMAs stream, so the
            # HAM clock gate is at 8/8 before the first projection and exp#1
            # isn't paying cold-clock prices (~3.4us of sustained PE activity
            # flips the gate)
            wup = psAV.tile([P, 512], F32, name="wup", tag="av")
            for i in range(18):
                nc.tensor.matmul(
                    wup[:], vt_sb[6][:, 0:P], vt_sb[7][:, 0:512],
                    start=True, stop=True,
                )

            # ---- projection helpers ----
            def qk_proj_cols(ct, dst, wb, srcb, nkt, bias_sb, c0, c1):
                """dst[:, c0:c1] = (W @ src)[ct*P:(ct+1)*P, c0:c1] + bias."""
                ps = psQK.tile([P, 512], F32, name="ps", tag="psqk")
                for kt in range(nkt):
                    nc.tensor.matmul(
                        ps[:, 0:c1 - c0],
                        wb[:, kt, ct * P:(ct + 1) * P],
                        srcb[:, kt, c0:c1],
                        start=(kt == 0),
                        stop=(kt == nkt - 1),
                    )
                nc.vector.tensor_scalar_add(
                    dst[:, c0:c1], ps[:, 0:c1 - c0], bias_sb[:, ct:ct + 1]
                )

            # ---- VT projection: VT[n, c] = sum_k y[k, n] * wvT[k, c]  (+ bias row)
            # medium priority (above AV/out-proj, below the exp chain): every
            # AV cell reads all eight vt tiles, so VT completing late gates
            # the whole AV pipeline. Groups alternate two pools so the
            # slot-release chain (MMs -> DVE copy -> next group) overlaps.
            def vt_proj(nt):
                with tc.high_priority(offset=HIPRI // 2):
                    # psAV pool: idle until the first AV group (~35us), so VT
                    # double-buffers there without touching the exp-critical
                    # psQK pool
                    ps = psAV.tile([P, 512], F32, name="ps", tag="av")
                    for kt in range(CCT):
                        nc.tensor.matmul(
                            ps[:],
                            yb[:, kt, nt * P:(nt + 1) * P],
                            wvb[:, kt, :],
                            start=(kt == 0),
                            stop=False,
                        )
                    nc.tensor.matmul(ps[:], ones_r[:], bv_sb[:], start=False, stop=True)
                    # scatter into [h][0:64] slots (col h*65+64 stays 1.0)
                    nc.vector.tensor_copy(
                        out=vt_sb[nt].rearrange("p (h e) -> p h e", e=DH + 1)[:, :, 0:DH],
                        in_=ps.rearrange("p (h d) -> p h d", d=DH),
                    )

            # pair-0 prologue: K in three column chunks chasing the y DMAs,
            # so exp#1 only waits on y strip0
            with tc.high_priority(offset=HIPRI):
                qk_proj_cols(0, q_sb[0], wqb, xb, CT, bq_sb, 0, 512)
                qk_proj_cols(0, k_sb[0], wkb, yb, CCT, bk_sb, 0, P)
                qk_proj_cols(0, k_sb[0], wkb, yb, CCT, bk_sb, P, 512)
                qk_proj_cols(0, k_sb[0], wkb, yb, CCT, bk_sb, 512, 768)
                qk_proj_cols(0, q_sb[0], wqb, xb, CT, bq_sb, 512, N)
                qk_proj_cols(0, k_sb[0], wkb, yb, CCT, bk_sb, 768, N)

            # ---- attention ----
            qk_sb = [bigp.tile([P, N], BF16, name=f"qk{t}", tag=f"qk{t}")
                     for t in range(CT)]

            def ecol(hh, qt):
                return (qt // 4) * 1024 + hh * 512 + (qt % 4) * P

            def emit_av_bank(ph, hh, ets, bank, quirk_spread):
                """AV for one (head, 4-qt bank). Cell accumulation groups within
                one PSUM bank must be sequential (start=True clears has_written
                for the whole bank), so cells run kt-inner back-to-back."""
                h = 2 * ph + hh
                oa = oap.tile([P, 256], BF16, name="oa", tag="oa")
                with tc.high_priority(offset=HIPRI // 2):
                    av = psAV.tile([P, 512], F32, name="av", tag="av")
                    for qq in range(4):
                        qt = bank * 4 + qq
                        for kt in range(QT):
                            nc.tensor.matmul(
                                av[:, qq * P:qq * P + DH + 1],
                                ets[kt][:, ecol(hh, qt):ecol(hh, qt) + P],
                                vt_sb[kt][:, h * (DH + 1):(h + 1) * (DH + 1)],
                                start=(kt == 0),
                                stop=(kt == QT - 1),
                            )
                # batched normalize: one reciprocal for the bank's 4
                # denominators, then per-cell scale + shuffle. Medium
                # priority: the psAV slot only frees after these reads, so
                # they pace the whole AV chain.
                with tc.high_priority(offset=HIPRI // 2):
                    rcp = rcpp.tile([P, 4], F32, name="rcp", tag="rcp")
                    nc.vector.reciprocal(
                        rcp[:], av.rearrange("p (q c) -> p q c", c=P)[:, :, DH]
                    )
                    for qq in range(4):
                        nc.vector.tensor_scalar_mul(
                            oa[:, qq * DH:(qq + 1) * DH],
                            av[:, qq * P:qq * P + DH], rcp[:, qq:qq + 1],
                        )
                # quirk shuffle:
                # qk[ph][hh*64 + qt*8 + p//16, 64*(p%16)+d] = O_h[qt*128+p, d]
                # pair 3's quirks all go to the scalar queue (its exp stream is
                # ending then); earlier pairs keep the sync queue, which is
                # idle once the input DMAs finish
                for qq in range(4):
                    qt = bank * 4 + qq
                    eng = nc.scalar if quirk_spread else nc.sync
                    eng.dma_start(
                        qk_sb[ph][hh * 64 + qt * 8: hh * 64 + qt * 8 + 8, :],
                        oa[:, qq * DH:(qq + 1) * DH],
                    )

            for ph in range(NH // 2):
                ets = [etp.tile([P, 2048], name="et", tag="et", dtype=BF16)
                       for _ in range(QT)]
                last_pair = ph == NH // 2 - 1
                for half in range(2):
                    for kt in range(QT):
                        with tc.high_priority(offset=HIPRI):
                            sps = psS.tile([P, 1024], F32, name="sps", tag="pss")
                            for hh in range(2):
                                bp = hh * DH
                                nc.tensor.matmul(
                                    sps[:, hh * 512:(hh + 1) * 512],
                                    k_sb[ph][bp:bp + DH, kt * P:(kt + 1) * P],
                                    q_sb[ph][bp:bp + DH, half * 512:(half + 1) * 512],
                                    start=True,
                                    stop=True,
                                )
                            nc.scalar.activation(
                                ets[kt][:, half * 1024:(half + 1) * 1024], sps[:],
                                EXP, scale=0.125,
                            )
                        if ph == 0 and half == 0:
                            vt_proj(kt)  # before first reader (AV below)
                        # next-pair projections gate the next pair's exp chain
                        if ph + 1 < NH // 2:
                            np1 = ph + 1
                            u = half * 8 + kt
                            if u == 1:
                                with tc.high_priority(offset=HIPRI):
                                    qk_proj_cols(np1, q_sb[np1], wqb, xb, CT, bq_sb, 0, 512)
                            elif u == 3:
                                with tc.high_priority(offset=HIPRI):
                                    qk_proj_cols(np1, k_sb[np1], wkb, yb, CCT, bk_sb, 0, 512)
                            elif u == 9:
                                with tc.high_priority(offset=HIPRI):
                                    qk_proj_cols(np1, q_sb[np1], wqb, xb, CT, bq_sb, 512, N)
                            elif u == 11:
                                with tc.high_priority(offset=HIPRI):
                                    qk_proj_cols(np1, k_sb[np1], wkb, yb, CCT, bk_sb, 512, N)
                    # bank `half` only needs the ETs of this half: emit right
                    # after the half's last unit so it overlaps the other half
                    spread = ph == 3
                    emit_av_bank(ph, 0, ets, half, quirk_spread=spread)
                    emit_av_bank(ph, 1, ets, half, quirk_spread=spread)

            # ---- output projection ----
            # alternate groups between psQK (free during pair 3) and psS
            # (free after the last exps) so consecutive groups pipeline
            # PE-matmul / DVE-bias / DMA instead of serializing on one slot
            out3 = out.rearrange("(t p) n -> p t n", p=P)
            groups = [(ct, half) for half in range(2) for ct in range(CT)]
            for gi, (ct, half) in enumerate(groups):
                pool, tag = [(psQK, "psqk"), (psS, "pss")][gi % 2]
                ps = pool.tile([P, 512], F32, name="ps", tag=tag)
                for kt in range(CT):
                    nc.tensor.matmul(
                        ps[:, 0:512],
                        wob[:, kt, ct * P:(ct + 1) * P],
                        qk_sb[kt][:, half * 512:(half + 1) * 512],
                        start=(kt == 0),
                        stop=(kt == CT - 1),
                    )
                ev = evp.tile([P, 512], BF16, name="ev", tag="ev")
                nc.vector.tensor_scalar_add(ev[:], ps[:, 0:512], bo_sb[:, ct:ct + 1])
                eng = nc.scalar if gi % 2 == 1 else nc.sync
                eng.dma_start(out3[:, ct, half * 512:(half + 1) * 512], ev[:])

    nc.compile()
    return nc


def kernel(**inputs) -> np.ndarray:
    bf = ml_dtypes.bfloat16
    x = np.ascontiguousarray(np.asarray(inputs["x"], dtype=np.float32).astype(bf))
    y = np.ascontiguousarray(np.asarray(inputs["y"], dtype=np.float32).astype(bf))
    wqT = np.ascontiguousarray(np.asarray(inputs["w_q"], dtype=np.float32).T.astype(bf))
    wkT = np.ascontiguousarray(np.asarray(inputs["w_k"], dtype=np.float32).T.astype(bf))
    wvT = np.ascontiguousarray(np.asarray(inputs["w_v"], dtype=np.float32).T.astype(bf))
    woT = np.ascontiguousarray(np.asarray(inputs["w_o"], dtype=np.float32).T.astype(bf))
    bq = np.ascontiguousarray(np.asarray(inputs["b_q"], dtype=np.float32))
    bk = np.ascontiguousarray(np.asarray(inputs["b_k"], dtype=np.float32))
    bv = np.ascontiguousarray(np.asarray(inputs["b_v"], dtype=np.float32).astype(bf))
    bo = np.ascontiguousarray(np.asarray(inputs["b_o"], dtype=np.float32))

    if "nc" not in _CACHE:
        _CACHE["nc"] = _build_nc()
    nc = _CACHE["nc"]

    in_maps = []
    for b in range(B):
        in_maps.append({
            "x": np.ascontiguousarray(x[b].reshape(C, N)),
            "y": np.ascontiguousarray(y[b].reshape(CC, N)),
            "wqT": wqT, "wkT": wkT, "wvT": wvT, "woT": woT,
            "bq": bq, "bk": bk, "bv": bv, "bo": bo,
        })
    res = run_bass_kernel_spmd(nc, in_maps, core_ids=list(range(B)))
    return np.stack([
        np.asarray(res.results[b]["out"]).astype(np.float32).reshape(C, HW, HW)
        for b in range(B)
    ])


# revision 35
# speedup vs baseline: 1.2512x; 1.0062x over previous
"""CrossAttention2D Trainium2 kernel (v4).

Sharding: data-parallel over batch. B=8 -> one batch element per NeuronCore,
no collectives. Weights replicated; host pre-transposes and casts to bf16.

Per-core math (C=512, Ccross=768, N=1024, 8 heads x 64):
  Q = Wq @ x_b          [C, N]   bf16
  K = Wk @ y_b          [C, N]   bf16
  VTa = [(Wv @ y_b).T | 1]       [N, 8*(64+1)] bf16 (ones col per head)
  per head pair ph (heads at PE rows 0/64, row-tiled scores):
    S[k, q] = K_h^T Q_h          psum [128, 1024] per (half, kt), ping-pong
    ET = exp(S/8)                ACT -> SBUF bf16, resident for whole pair
    O_aug[q, 0:65] = ET_tile^T @ VTa_h   (ET stationary, 65-col streams,
                                          kt-inner per cell; col 64 = denom)
    O = O_aug[:, :64] / O_aug[:, 64]     (DVE reciprocal + scale) -> bf16
  quirk: out_flat[h*64 + q//16, 64*(q%16) + d] = O_h[q, d]  (DMA shuffle)
  out = Wo @ quirk + bo          [C, N] -> bf16 out, host casts to f32

v4 scheduling (from v2/v3 ntff traces):
  - separate PSUM pools for next-pair Q/K proj (psQK) vs VT proj (psVT):
    in v2/v3 they shared one 2-slot pool, so the exp-critical Q/K proj
    serialized behind low-priority VT work -> 5-14us ACT stall per pair
  - exp-critical chain (Q/K proj, scores, exp) under tc.high_priority
  - consolidated 3D input DMAs (one instr per tensor slice group, ~600ns
    issue each) ordered so exp#1 only waits on ~1MB: wq-ct0, x-h0,
    wk-ct0, y-strip0; K proj for pair 0 follows the slices
  - AV emitted per (pair, bank): bank0 only needs half0 ETs
  - last pair's quirk + output DMAs alternate sync/scalar (ACT idle)
  - out-proj groups ct0/ct1 use psQK/psVT (free during pairs 1-3 -> they
    pre-run), ct2/ct3 use psS (free after the last exp)
"""

import numpy as np
import ml_dtypes

import concourse.bass as bass
import concourse.mybir as mybir
import concourse.tile as tile
from concourse import bacc
from concourse.bass_utils import run_bass_kernel_spmd

P = 128
C = 512          # d_embed
CC = 768         # d_cross
N = 1024         # H*W = 32*32
NH = 8
DH = 64
CT = C // P      # 4
CCT = CC // P    # 6
QT = N // P      # 8
HW = 32
B = 8
F32 = mybir.dt.float32
BF16 = mybir.dt.bfloat16
HIPRI = 1_000_000

_CACHE = {}


def _build_nc():
    nc = bacc.Bacc("TRN2", target_bir_lowering=False, debug=False, num_devices=B)

    x = nc.dram_tensor("x", [C, N], BF16, kind="ExternalInput")
    y = nc.dram_tensor("y", [CC, N], BF16, kind="ExternalInput")
    wqT = nc.dram_tensor("wqT", [C, C], BF16, kind="ExternalInput")
    wkT = nc.dram_tensor("wkT", [CC, C], BF16, kind="ExternalInput")
    wvT = nc.dram_tensor("wvT", [CC, C], BF16, kind="ExternalInput")
    woT = nc.dram_tensor("woT", [C, C], BF16, kind="ExternalInput")
    bq = nc.dram_tensor("bq", [C], F32, kind="ExternalInput")
    bk = nc.dram_tensor("bk", [C], F32, kind="ExternalInput")
    bv = nc.dram_tensor("bv", [C], BF16, kind="ExternalInput")
    bo = nc.dram_tensor("bo", [C], F32, kind="ExternalInput")
    out = nc.dram_tensor("out", [C, N], BF16, kind="ExternalOutput")

    EXP = mybir.ActivationFunctionType.Exp

    with tile.TileContext(nc) as tc:
        with (
            tc.tile_pool(name="const", bufs=1) as constp,
            tc.tile_pool(name="big", bufs=1) as bigp,
            tc.tile_pool(name="et", bufs=28) as etp,
            tc.tile_pool(name="oa", bufs=8) as oap,
            tc.tile_pool(name="ev", bufs=3) as evp,
            tc.tile_pool(name="rcp", bufs=4) as rcpp,
            tc.tile_pool(name="psS", bufs=2, space="PSUM") as psS,
            tc.tile_pool(name="psAV", bufs=3, space="PSUM") as psAV,
            tc.tile_pool(name="psQK", bufs=1, space="PSUM") as psQK,
        ):
            # ---- constants ----
            ones_r = constp.tile([1, P], BF16, name="ones_r", tag="ones_r")
            nc.vector.memset(ones_r[:], 1.0)
            # preload the exp table set early so the ~2.7us ACT_TABLE_LOAD
            # overlaps the input DMA phase instead of the first real exp
            dmy = constp.tile([P, 1], F32, name="dmy", tag="dmy")
            nc.vector.memset(dmy[:], 0.0)
            dmy2 = constp.tile([P, 1], F32, name="dmy2", tag="dmy2")
            nc.scalar.activation(dmy2[:], dmy[:], EXP)

            bq_sb = constp.tile([P, CT], F32, name="bq", tag="bq")
            bk_sb = constp.tile([P, CT], F32, name="bk", tag="bk")
            bo_sb = constp.tile([P, CT], F32, name="bo", tag="bo")
            bv_sb = constp.tile([1, C], BF16, name="bv", tag="bv")

            # ---- consolidated input tiles (3D views) ----
            x3 = x.rearrange("(t p) n -> p t n", p=P)
            y3 = y.rearrange("(t p) n -> p t n", p=P)
            wq3 = wqT.rearrange("(t p) m -> p t m", p=P)
            wk3 = wkT.rearrange("(t p) m -> p t m", p=P)
            wv3 = wvT.rearrange("(t p) m -> p t m", p=P)
            wo3 = woT.rearrange("(t p) m -> p t m", p=P)

            xb = bigp.tile([P, CT, N], BF16, name="xb", tag="xb")
            yb = bigp.tile([P, CCT, N], BF16, name="yb", tag="yb")
            wqb = bigp.tile([P, CT, C], BF16, name="wqb", tag="wqb")
            wkb = bigp.tile([P, CCT, C], BF16, name="wkb", tag="wkb")
            wvb = bigp.tile([P, CCT, C], BF16, name="wvb", tag="wvb")
            wob = bigp.tile([P, CT, C], BF16, name="wob", tag="wob")

            # all input DMAs on ONE queue (sync) in strict priority order:
            # the DGE issue order is also the HBM bandwidth allocation order,
            # so eager second-wave DMAs must not race the exp#1-critical wave
            # (v4 lost ~5us to x-h0 sharing bandwidth with later loads)
            nc.scalar.dma_start(bq_sb[:], bq.rearrange("(o p) -> p o", p=P))
            nc.scalar.dma_start(bk_sb[:], bk.rearrange("(o p) -> p o", p=P))
            nc.sync.dma_start(wqb[:, :, 0:P], wq3[:, :, 0:P])
            nc.sync.dma_start(xb[:, :, 0:512], x3[:, :, 0:512])
            nc.sync.dma_start(wkb[:, :, 0:P], wk3[:, :, 0:P])
            nc.sync.dma_start(yb[:, :, 0:P], y3[:, :, 0:P])
            # second wave, in order of first use (wv early: the VT chain
            # gates the whole AV pipeline; y half1 split so K cols 512-767
            # are projectable sooner)
            nc.sync.dma_start(yb[:, :, P:512], y3[:, :, P:512])
            nc.sync.dma_start(yb[:, :, 512:768], y3[:, :, 512:768])
            nc.sync.dma_start(yb[:, :, 768:N], y3[:, :, 768:N])
            nc.sync.dma_start(wvb[:], wv3[:])
            nc.sync.dma_start(xb[:, :, 512:N], x3[:, :, 512:N])
            nc.sync.dma_start(wqb[:, :, P:C], wq3[:, :, P:C])
            nc.sync.dma_start(wkb[:, :, P:C], wk3[:, :, P:C])
            nc.sync.dma_start(wob[:], wo3[:])
            nc.sync.dma_start(bo_sb[:], bo.rearrange("(o p) -> p o", p=P))
            nc.sync.dma_start(bv_sb[:], bv[None, :])

            q_sb = [bigp.tile([P, N], BF16, name=f"q{t}", tag=f"q{t}") for t in range(CT)]
            k_sb = [bigp.tile([P, N], BF16, name=f"k{t}", tag=f"k{t}") for t in range(CT)]
            # VTa buffer: per n-tile, cols laid out [h][65] with col h*65+64 == 1.0
            vt_sb = [bigp.tile([P, NH * (DH + 1)], BF16, name=f"vt{t}", tag=f"vt{t}")
                     for t in range(QT)]
            for t in range(QT):
                nc.gpsimd.memset(vt_sb[t][:], 1.0)

            # PE warm-up: dummy matmuls while the input DMAs stream, so the
            # HAM clock gate is at 8/8 before the first projection and exp#1
            # isn't paying cold-clock prices (~3.4us of sustained PE activity
            # flips the gate)
            wup = psAV.tile([P, 512], F32, name="wup", tag="av")
            for i in range(18):
                nc.tensor.matmul(
                    wup[:], vt_sb[6][:, 0:P], vt_sb[7][:, 0:512],
                    start=True, stop=True,
                )

            # ---- projection helpers ----
            def qk_proj_cols(ct, dst, wb, srcb, nkt, bias_sb, c0, c1, pool=None):
                """dst[:, c0:c1] = (W @ src)[ct*P:(ct+1)*P, c0:c1] + bias."""
                if pool is None:
                    pool, tag = psQK, "psqk"
                else:
                    pool, tag = pool
                ps = pool.tile([P, 512], F32, name="ps", tag=tag)
                for kt in range(nkt):
                    nc.tensor.matmul(
                        ps[:, 0:c1 - c0],
                        wb[:, kt, ct * P:(ct + 1) * P],
                        srcb[:, kt, c0:c1],
                        start=(kt == 0),
                        stop=(kt == nkt - 1),
                    )
                nc.vector.tensor_scalar_add(
                    dst[:, c0:c1], ps[:, 0:c1 - c0], bias_sb[:, ct:ct + 1]
                )

            # ---- VT projection: VT[n, c] = sum_k y[k, n] * wvT[k, c]  (+ bias row)
            # medium priority (above AV/out-proj, below the exp chain): every
            # AV cell reads all eight vt tiles, so VT completing late gates
            # the whole AV pipeline. Groups alternate two pools so the
            # slot-release chain (MMs -> DVE copy -> next group) overlaps.
            def vt_proj(nt):
                with tc.high_priority(offset=HIPRI // 2):
                    # psAV pool: idle until the first AV group (~35us), so VT
                    # double-buffers there without touching the exp-critical
                    # psQK pool
                    ps = psAV.tile([P, 512], F32, name="ps", tag="av")
                    for kt in range(CCT):
                        nc.tensor.matmul(
                            ps[:],
                            yb[:, kt, nt * P:(nt + 1) * P],
                            wvb[:, kt, :],
                            start=(kt == 0),
                            stop=False,
                        )
                    nc.tensor.matmul(ps[:], ones_r[:], bv_sb[:], start=False, stop=True)
                    # scatter into [h][0:64] slots (col h*65+64 stays 1.0)
                    nc.vector.tensor_copy(
                        out=vt_sb[nt].rearrange("p (h e) -> p h e", e=DH + 1)[:, :, 0:DH],
                        in_=ps.rearrange("p (h d) -> p h d", d=DH),
                    )

            # pair-0 prologue: K in three column chunks chasing the y DMAs,
            # so exp#1 only waits on y strip0
            # prologue alternates psQK with the (still unused) psS slots so
            # the six chunks form two independent slot chains, not one
            with tc.high_priority(offset=HIPRI):
                qk_proj_cols(0, q_sb[0], wqb, xb, CT, bq_sb, 0, 512)
                qk_proj_cols(0, k_sb[0], wkb, yb, CCT, bk_sb, 0, P, pool=(psS, "pss"))
                qk_proj_cols(0, k_sb[0], wkb, yb, CCT, bk_sb, P, 512)
                qk_proj_cols(0, k_sb[0], wkb, yb, CCT, bk_sb, 512, 768, pool=(psS, "pss"))
                qk_proj_cols(0, q_sb[0], wqb, xb, CT, bq_sb, 512, N)
                qk_proj_cols(0, k_sb[0], wkb, yb, CCT, bk_sb, 768, N, pool=(psS, "pss"))

            # ---- attention ----
            qk_sb = [bigp.tile([P, N], BF16, name=f"qk{t}", tag=f"qk{t}")
                     for t in range(CT)]

            def ecol(hh, qt):
                return (qt // 4) * 1024 + hh * 512 + (qt % 4) * P

            def emit_av_bank(ph, hh, ets, bank, quirk_spread):
                """AV for one (head, 4-qt bank). Cell accumulation groups within
                one PSUM bank must be sequential (start=True clears has_written
                for the whole bank), so cells run kt-inner back-to-back."""
                h = 2 * ph + hh
                oa = oap.tile([P, 256], BF16, name="oa", tag="oa")
                with tc.high_priority(offset=HIPRI // 2):
                    av = psAV.tile([P, 512], F32, name="av", tag="av")
                    for qq in range(4):
                        qt = bank * 4 + qq
                        for kt in range(QT):
                            nc.tensor.matmul(
                                av[:, qq * P:qq * P + DH + 1],
                                ets[kt][:, ecol(hh, qt):ecol(hh, qt) + P],
                                vt_sb[kt][:, h * (DH + 1):(h + 1) * (DH + 1)],
                                start=(kt == 0),
                                stop=(kt == QT - 1),
                            )
                # batched normalize: one reciprocal for the bank's 4
                # denominators, then per-cell scale + shuffle. Medium
                # priority: the psAV slot only frees after these reads, so
                # they pace the whole AV chain.
                with tc.high_priority(offset=HIPRI // 2):
                    rcp = rcpp.tile([P, 4], F32, name="rcp", tag="rcp")
                    nc.vector.reciprocal(
                        rcp[:], av.rearrange("p (q c) -> p q c", c=P)[:, :, DH]
                    )
                    for qq in range(4):
                        nc.vector.tensor_scalar_mul(
                            oa[:, qq * DH:(qq + 1) * DH],
                            av[:, qq * P:qq * P + DH], rcp[:, qq:qq + 1],
                        )
                # quirk shuffle:
                # qk[ph][hh*64 + qt*8 + p//16, 64*(p%16)+d] = O_h[qt*128+p, d]
                # pair 3's quirks all go to the scalar queue (its exp stream is
                # ending then); earlier pairs keep the sync queue, which is
                # idle once the input DMAs finish
                for qq in range(4):
                    qt = bank * 4 + qq
                    eng = nc.scalar if (quirk_spread and qq % 2 == 1) else nc.sync
                    eng.dma_start(
                        qk_sb[ph][hh * 64 + qt * 8: hh * 64 + qt * 8 + 8, :],
                        oa[:, qq * DH:(qq + 1) * DH],
                    )

            for ph in range(NH // 2):
                ets = [etp.tile([P, 2048], name="et", tag="et", dtype=BF16)
                       for _ in range(QT)]
                last_pair = ph == NH // 2 - 1
                for half in range(2):
                    for kt in range(QT):
                        with tc.high_priority(offset=HIPRI):
                            sps = psS.tile([P, 1024], F32, name="sps", tag="pss")
                            for hh in range(2):
                                bp = hh * DH
                                nc.tensor.matmul(
                                    sps[:, hh * 512:(hh + 1) * 512],
                                    k_sb[ph][bp:bp + DH, kt * P:(kt + 1) * P],
                                    q_sb[ph][bp:bp + DH, half * 512:(half + 1) * 512],
                                    start=True,
                                    stop=True,
                                )
                            nc.scalar.activation(
                                ets[kt][:, half * 1024:(half + 1) * 1024], sps[:],
                                EXP, scale=0.125,
                            )
                        if ph == 0 and half == 0:
                            vt_proj(kt)  # before first reader (AV below)
                        # next-pair projections gate the next pair's exp chain
                        if ph + 1 < NH // 2:
                            np1 = ph + 1
                            u = half * 8 + kt
                            if u == 1:
                                with tc.high_priority(offset=HIPRI):
                                    qk_proj_cols(np1, q_sb[np1], wqb, xb, CT, bq_sb, 0, 512)
                            elif u == 3:
                                with tc.high_priority(offset=HIPRI):
                                    qk_proj_cols(np1, k_sb[np1], wkb, yb, CCT, bk_sb, 0, 512)
                            elif u == 9:
                                with tc.high_priority(offset=HIPRI):
                                    qk_proj_cols(np1, q_sb[np1], wqb, xb, CT, bq_sb, 512, N)
                            elif u == 11:
                                with tc.high_priority(offset=HIPRI):
                                    qk_proj_cols(np1, k_sb[np1], wkb, yb, CCT, bk_sb, 512, N)
                    # bank `half` only needs the ETs of this half: emit right
                    # after the half's last unit so it overlaps the other half
                    spread = ph >= 2
                    emit_av_bank(ph, 0, ets, half, quirk_spread=spread)
                    emit_av_bank(ph, 1, ets, half, quirk_spread=spread)

            # ---- output projection ----
            # alternate groups between psQK (free during pair 3) and psS
            # (free after the last exps) so consecutive groups pipeline
            # PE-matmul / DVE-bias / DMA instead of serializing on one slot
            out3 = out.rearrange("(t p) n -> p t n", p=P)
            groups = [(ct, half) for half in range(2) for ct in range(CT)]
            for gi, (ct, half) in enumerate(groups):
                pool, tag = [(psQK, "psqk"), (psS, "pss")][gi % 2]
                ps = pool.tile([P, 512], F32, name="ps", tag=tag)
                for kt in range(CT):
                    nc.tensor.matmul(
                        ps[:, 0:512],
                        wob[:, kt, ct * P:(ct + 1) * P],
                        qk_sb[kt][:, half * 512:(half + 1) * 512],
                        start=(kt == 0),
                        stop=(kt == CT - 1),
                    )
                ev = evp.tile([P, 512], BF16, name="ev", tag="ev")
                nc.vector.tensor_scalar_add(ev[:], ps[:, 0:512], bo_sb[:, ct:ct + 1])
                eng = nc.scalar if gi % 2 == 1 else nc.sync
                eng.dma_start(out3[:, ct, half * 512:(half + 1) * 512], ev[:])

    nc.compile()
    return nc


def kernel(**inputs) -> np.ndarray:
    bf = ml_dtypes.bfloat16
    x = np.ascontiguousarray(np.asarray(inputs["x"], dtype=np.float32).astype(bf))
    y = np.ascontiguousarray(np.asarray(inputs["y"], dtype=np.float32).astype(bf))
    wqT = np.ascontiguousarray(np.asarray(inputs["w_q"], dtype=np.float32).T.astype(bf))
    wkT = np.ascontiguousarray(np.asarray(inputs["w_k"], dtype=np.float32).T.astype(bf))
    wvT = np.ascontiguousarray(np.asarray(inputs["w_v"], dtype=np.float32).T.astype(bf))
    woT = np.ascontiguousarray(np.asarray(inputs["w_o"], dtype=np.float32).T.astype(bf))
    bq = np.ascontiguousarray(np.asarray(inputs["b_q"], dtype=np.float32))
    bk = np.ascontiguousarray(np.asarray(inputs["b_k"], dtype=np.float32))
    bv = np.ascontiguousarray(np.asarray(inputs["b_v"], dtype=np.float32).astype(bf))
    bo = np.ascontiguousarray(np.asarray(inputs["b_o"], dtype=np.float32))

    if "nc" not in _CACHE:
        _CACHE["nc"] = _build_nc()
    nc = _CACHE["nc"]

    in_maps = []
    for b in range(B):
        in_maps.append({
            "x": np.ascontiguousarray(x[b].reshape(C, N)),
            "y": np.ascontiguousarray(y[b].reshape(CC, N)),
            "wqT": wqT, "wkT": wkT, "wvT": wvT, "woT": woT,
            "bq": bq, "bk": bk, "bv": bv, "bo": bo,
        })
    res = run_bass_kernel_spmd(nc, in_maps, core_ids=list(range(B)))
    return np.stack([
        np.asarray(res.results[b]["out"]).astype(np.float32).reshape(C, HW, HW)
        for b in range(B)
    ])
